# revision 1
# baseline (speedup 1.0000x reference)
"""Trainium2 Bass kernel for ViT-style attention with relative position bias.

Module (per batch b, head h):
    qkv = x @ qkv_w.T + cat(q_bias, 0, v_bias)
    attn = softmax(scale * q @ k.T + bias[h])          bias = rel_pos_table[rel_pos_index]
    out  = (attn @ v) @ proj_w.T + proj_b

Distribution: pure data-parallel over batch — 8 NeuronCores x 8 batches each,
no collectives. Each core runs an identical SPMD program on its batch shard.

Device-side layout strategy (per core, all intermediates SBUF-resident bf16):
  - Host packs every input as its exact SBUF image so each tensor loads with
    one (or a few) large fully-contiguous DMAs, ordered by first use:
    xT+wv first (v_block deps), then wqk per head-pair, expb, wp.
  - While the first DMAs are in flight the PE runs warm-up matmuls on a junk
    tile so the HAM clock gate is at 2.4 GHz when real work starts.
  - qk^T matmul produces Q^T/K^T feature-major [64, tokens] slices directly,
    m-tile order interleaved [Q0,K0,Q1,K1,...] to match the DMA chunks.
  - V matmul produces V token-major [tokens, 64] (the AV contraction needs
    keys on the partition axis), with a ones column block per head so the
    AV matmul also emits the softmax denominator rows for free.
  - Scores are computed transposed, S^T[j, i] = K[j] . Q[i]; softmax uses
    exp(s) * exp(bias) (no max-subtraction: logits are O(3), fp32/bf16 safe);
    exp(bias) is an input-derived constant computed host-side.
  - AV^T [64+64, 197] = [V_h | ones]^T @ expS^T is feature-major, feeding the
    proj matmul without transposes; the replicated denominator lands in
    partitions 64:128 and is reciprocal'd straight out of PSUM.
  - Elementwise work is pinned per-engine (EXP: scalar, expb-mul mostly
    gpsimd, recip+normalize: vector, evacs balanced) so no engine exceeds
    the PE's per-block budget.
"""

import os
import numpy as np
import ml_dtypes

import concourse.bass as bass
import concourse.bacc as bacc
import concourse.mybir as mybir
import concourse.tile as tile
from concourse import bass_utils

F32 = mybir.dt.float32
BF16 = mybir.dt.bfloat16

N_CORES = 8
B = 64
B_LOC = B // N_CORES          # 8 batches per core
N = 197                       # tokens per batch
C = 768
H = 12
HD = 64
SCALE = HD ** -0.5
NTOK = B_LOC * N              # 1576
NPAD = 1664                   # 13 * 128 (qkT/aoT column allocation)
KT = 6                        # 768 / 128 contraction tiles
QK_NT = 4                     # n-tiles over tokens
QK_TW = NTOK // QK_NT         # 394

_CACHE = {}
WARMUP = int(os.environ.get("K_WARMUP", "30"))
BIGDMA = os.environ.get("K_BIGDMA", "1") == "1"
OUTF32 = os.environ.get("K_OUTF32", "0") == "1"
VECMEMSET = os.environ.get("K_VECMEMSET", "0") == "1"



def _build(has_v_bias, has_p_bias, has_q_bias):
    nc = bacc.Bacc("TRN2", target_bir_lowering=False, debug=False)

    xT_d = nc.dram_tensor("xT", [128, QK_NT, KT, QK_TW], BF16, kind="ExternalInput")
    wqk_d = nc.dram_tensor("wqk", [128, 12, KT, 128], BF16, kind="ExternalInput")
    wv_d = nc.dram_tensor("wv", [128, KT, C], BF16, kind="ExternalInput")
    wp_d = nc.dram_tensor("wp", [128, KT, C], BF16, kind="ExternalInput")
    expb_d = nc.dram_tensor("expb", [128, H // 2, 4 * N], BF16, kind="ExternalInput")
    if has_q_bias:
        qb_d = nc.dram_tensor("qb", [128, KT], F32, kind="ExternalInput")
    if has_v_bias:
        vb_d = nc.dram_tensor("vb", [1, C], BF16, kind="ExternalInput")
    if has_p_bias:
        pb_d = nc.dram_tensor("pb", [1, C], BF16, kind="ExternalInput")
    out_d = nc.dram_tensor("out", [NTOK, C], F32 if OUTF32 else BF16, kind="ExternalOutput")

    with tile.TileContext(nc) as tc:
        with (
            tc.tile_pool(name="singles", bufs=1) as singles,
            tc.tile_pool(name="expwork", bufs=4) as expwork,
            tc.tile_pool(name="normwork", bufs=3) as normwork,
            tc.tile_pool(name="outstage", bufs=3) as outstage,
        ):
            # ---- persistent SBUF tensors ----
            wp_sb = singles.tile([128, KT, C], BF16)
            expb_sb = singles.tile([128, H // 2, 4 * N], BF16)
            # m-tile order [Q0, K0, Q1, K1, ...] to match wqk DMA chunks
            qkT_sb = singles.tile([128, 12, NPAD], BF16)
            # per-head blocks [V_h 64 | ones 64]; the ones half makes each AV
            # matmul replicate the softmax denominator into PSUM partitions
            # 64:128 (lhsT = [V_h | ones], 128 contiguous columns)
            V_sb = singles.tile([128, 2 * B_LOC, H, 2 * HD], BF16)
            aoT_sb = singles.tile([128, KT, NPAD], BF16)      # attn-out^T, proj stationary
            junk_sb = singles.tile([128, 512], BF16)

            if has_q_bias:
                qb_sb = singles.tile([128, KT], F32)
                nc.sync.dma_start(out=qb_sb, in_=qb_d.ap())
            if has_v_bias or has_p_bias:
                ones_row = singles.tile([1, NPAD], BF16)
                nc.vector.memset(ones_row, 1.0)
            if has_p_bias:
                pb_sb = singles.tile([1, C], BF16)
                nc.sync.dma_start(out=pb_sb, in_=pb_d.ap())

            # warm-up fodder + ones blocks for the AV denominator trick
            nc.vector.memset(junk_sb, 1.0)
            for s in range(2 * B_LOC):
                nc.vector.memset(V_sb[:, s, :, HD:2 * HD], 1.0)
            # proj reads padded token columns of attn-out^T; keep them finite
            nc.vector.memset(aoT_sb[:, :, NTOK:NPAD], 0.0)
            # score lhsT for the last batch reads K-mtile columns past NTOK
            ms_eng = nc.vector if VECMEMSET else nc.gpsimd
            for m in range(12):
                if m % 2 == 1:
                    ms_eng.memset(qkT_sb[:, m, NTOK:NPAD], 0.0)

            # ---- merged qk^T + V + attention, interleaved per head pair ----
            with tc.tile_pool(name="wqkpool", bufs=1) as wqkpool:
                wqk_sb = wqkpool.tile([128, 12, KT, 128], BF16)
                xT_sb = wqkpool.tile([128, QK_NT, KT, QK_TW], BF16)

                with (
                    tc.tile_pool(name="ps_qk", bufs=2, space="PSUM") as ps_qk,
                    tc.tile_pool(name="ps_s", bufs=1, space="PSUM") as ps_s_pool,
                    tc.tile_pool(name="ps_av2", bufs=2, space="PSUM") as ps_av_pool,
                ):
                    evac_n = [0]
                    # gpsimd cannot access PSUM — evacs go scalar/vector only
                    evac_engines = [nc.scalar, nc.vector]

                    def evac_copy(out, in_):
                        e = evac_engines[evac_n[0] % len(evac_engines)]
                        evac_n[0] += 1
                        if e is nc.scalar:
                            e.copy(out=out, in_=in_)
                        else:
                            e.tensor_copy(out=out, in_=in_)

                    def qk_piece(mt, nt):
                            pq = ps_qk.tile([128, QK_TW], F32, name="psqk", tag="psqk")
                            for kt in range(KT):
                                nc.tensor.matmul(
                                    pq,
                                    lhsT=wqk_sb[:, mt, kt, :],
                                    rhs=xT_sb[:, nt, kt, :],
                                    start=(kt == 0), stop=(kt == KT - 1),
                                )
                            dst = qkT_sb[:, mt, nt * QK_TW:(nt + 1) * QK_TW]
                            if has_q_bias and mt % 2 == 0:
                                nc.any.tensor_scalar_add(dst, pq, qb_sb[:, mt // 2:mt // 2 + 1])
                            else:
                                nc.vector.tensor_copy(out=dst, in_=pq)

                    def qk_mtile(mt):
                        for nt in range(QK_NT):
                            qk_piece(mt, nt)

                    def v_block(b, wv_sb, vb_sb):
                        # token-major V for one batch, psum borrowed from ps_qk
                        for jc in range(2):
                            m = 128 if jc == 0 else N - 128
                            tok0 = b * N + jc * 128
                            tc_, to_ = tok0 // QK_TW, tok0 % QK_TW
                            for nt in range(2):
                                pv = ps_qk.tile([128, QK_TW], F32, name="psqk", tag="psqk")
                                for kt in range(KT):
                                    nc.tensor.matmul(
                                        pv[0:m, 0:C // 2],
                                        lhsT=xT_sb[:, tc_, kt, to_:to_ + m],
                                        rhs=wv_sb[:, kt, nt * (C // 2):(nt + 1) * (C // 2)],
                                        start=(kt == 0),
                                        stop=(kt == KT - 1 and not has_v_bias),
                                    )
                                if has_v_bias:
                                    nc.tensor.matmul(
                                        pv[0:m, 0:C // 2],
                                        lhsT=ones_row[:, tok0:tok0 + m],
                                        rhs=vb_sb[:, nt * (C // 2):(nt + 1) * (C // 2)],
                                        start=False, stop=True,
                                    )
                                evac_copy(
                                    V_sb[0:m, b * 2 + jc, nt * 6:(nt + 1) * 6, 0:HD],
                                    pv[0:m, 0:C // 2].rearrange("p (g f) -> p g f", g=KT),
                                )

                    def attn_scores(hp, b, mul_eng):
                        q0 = b * N
                        # Both heads' scores in one 2-bank PSUM tile (head hh at
                        # columns hh*512 + [0, 394)) so one bank-hopping AP
                        # covers the pair in a single EXP. Even/odd heads live
                        # at base partitions 0/64, so their k=64 score matmuls
                        # occupy disjoint PE row groups and overlap.
                        ps_s2 = ps_s_pool.tile([128, 1024], F32, name="ps_s2", tag="ps_s2")
                        for jc in range(2):
                            for hh in range(2):
                                h = hp * 2 + hh
                                pbase = (h % 2) * 64
                                nc.tensor.matmul(
                                    ps_s2[:, hh * 512 + jc * N:hh * 512 + (jc + 1) * N],
                                    lhsT=qkT_sb[pbase:pbase + 64, 2 * hp + 1,
                                                q0 + jc * 128:q0 + jc * 128 + 128],
                                    rhs=qkT_sb[pbase:pbase + 64, 2 * hp, q0:q0 + N],
                                    start=(jc == 0), stop=(jc == 1),
                                )
                        expS2 = expwork.tile([128, 4 * N], BF16, tag="expS2")
                        nc.scalar.activation(
                            expS2.rearrange("p (g w) -> p g w", w=2 * N),
                            ps_s2.rearrange("p (g w) -> p g w", w=512)[:, :, 0:2 * N],
                            mybir.ActivationFunctionType.Exp, scale=SCALE,
                        )
                        expST2 = expwork.tile([128, 4 * N], BF16, tag="expST2")
                        mul_eng.tensor_mul(expST2, expS2, expb_sb[:, hp, :])
                        return expST2

                    def attn_av(hp, b, av, expST2):
                        for hh in range(2):
                            h = hp * 2 + hh
                            for jc in range(2):
                                jm = 128 if jc == 0 else N - 128
                                nc.tensor.matmul(
                                    av[:, hh * N:(hh + 1) * N],
                                    lhsT=V_sb[0:jm, b * 2 + jc, h, :],
                                    rhs=expST2[0:jm, hh * 2 * N + jc * N:hh * 2 * N + (jc + 1) * N],
                                    start=(hh == 0 and jc == 0),
                                    stop=(hh == 1 and jc == 1),
                                )

                    back_n = [0]

                    def attn_back2(hp, b, av2):
                        # one copy/recip/mul set covers pairs (hp,b) and (hp,b+1)
                        denom2 = normwork.tile([64, 2, 2 * N], F32, tag="denom")
                        nc.scalar.copy(out=denom2, in_=av2[64:128, :, 0:2 * N])
                        back_n[0] += 1
                        recipB2 = normwork.tile([64, 2, 2 * N], F32, tag="recipB")
                        nc.vector.reciprocal_approx_fast(recipB2, denom2)
                        for hh in range(2):
                            h = hp * 2 + hh
                            dst = aoT_sb[(h % 2) * 64:(h % 2) * 64 + 64, h // 2,
                                         b * N:(b + 2) * N]
                            nc.vector.tensor_mul(
                                dst.rearrange("p (n w) -> p n w", n=2),
                                av2[0:HD, :, hh * N:(hh + 1) * N],
                                recipB2[:, :, hh * N:(hh + 1) * N],
                            )

                    out_dma = [nc.sync, nc.scalar]

                    def proj_mtile(mt):
                        rows = min(128, NTOK - mt * 128)
                        ps = [ps_qk.tile([128, QK_TW], F32, name="psqk", tag="psqk")
                              for _ in range(2)]
                        for kt in range(KT):
                            for nt in range(2):
                                nc.tensor.matmul(
                                    ps[nt][:, 0:C // 2],
                                    lhsT=aoT_sb[:, kt, mt * 128:(mt + 1) * 128],
                                    rhs=wp_sb[:, kt, nt * (C // 2):(nt + 1) * (C // 2)],
                                    start=(kt == 0),
                                    stop=(kt == KT - 1 and not has_p_bias),
                                )
                        if has_p_bias:
                            for nt in range(2):
                                nc.tensor.matmul(
                                    ps[nt][:, 0:C // 2],
                                    lhsT=ones_row[:, mt * 128:(mt + 1) * 128],
                                    rhs=pb_sb[:, nt * (C // 2):(nt + 1) * (C // 2)],
                                    start=False, stop=True,
                                )
                        stage = outstage.tile([128, C], F32 if OUTF32 else BF16, tag="stage")
                        nc.scalar.copy(out=stage[:, 0:C // 2], in_=ps[0][:, 0:C // 2])
                        nc.vector.tensor_copy(out=stage[:, C // 2:C], in_=ps[1][:, 0:C // 2])
                        out_dma[0 if OUTF32 else mt % 2].dma_start(
                            out=out_d.ap()[mt * 128:mt * 128 + rows, :], in_=stage[0:rows, :]
                        )

                    # proj m-tile mt is ready once its last token's batch has
                    # finished all heads; stream proj into the hp=5 block
                    proj_after = [[] for _ in range(B_LOC)]
                    for mt in range(NPAD // 128):
                        t_end = min((mt + 1) * 128, NTOK)
                        proj_after[(t_end - 1) // N].append(mt)

                    def on_backed2(hp, b):
                        if hp == H // 2 - 1:
                            for mt in proj_after[b] + proj_after[b + 1]:
                                proj_mtile(mt)

                    inflight = []   # av2 groups awaiting back2
                    pend = []       # scored pairs awaiting their AV matmuls

                    pair_n = [0]
                    # 3:1 gpsimd:vector for the exp(bias) multiply (SBUF-only op)
                    mul_cycle = [nc.gpsimd, nc.gpsimd, nc.vector, nc.gpsimd]

                    def flush_av():
                        # emit AV matmuls for the two OLDEST scored pairs —
                        # their expb-muls got a full group of slack, so the
                        # FIFO PE queue never head-blocks on the softmax chain
                        (h0, b0, e0), (h1, b1, e1) = pend.pop(0), pend.pop(0)
                        av2 = ps_av_pool.tile([128, 2, 512], F32, name="av2", tag="av2")
                        attn_av(h0, b0, av2[:, 0, 0:2 * N], e0)
                        attn_av(h1, b1, av2[:, 1, 0:2 * N], e1)
                        inflight.append((h0, b0, av2))

                    def flush_back():
                        done = inflight.pop(0)
                        attn_back2(*done)
                        on_backed2(done[0], done[1])

                    def do_group(hp, b, filler_fn):
                        for db in range(2):
                            mul_eng = mul_cycle[pair_n[0] % len(mul_cycle)]
                            pair_n[0] += 1
                            pend.append((hp, b + db, attn_scores(hp, b + db, mul_eng)))
                        # drain eagerly in the last block so proj work spreads
                        if len(inflight) >= (2 if hp < H // 2 - 1 else 1):
                            flush_back()
                        filler_fn()
                        if len(pend) >= 4:
                            flush_av()

                    with tc.tile_pool(name="xvpool", bufs=1) as xvpool:
                        wv_sb = xvpool.tile([128, KT, C], BF16)

                        # ---- input DMAs: one large contiguous burst each ----
                        for c in range(QK_NT):
                            nc.sync.dma_start(out=xT_sb[:, c], in_=xT_d.ap()[:, c])
                        nc.scalar.dma_start(out=wv_sb, in_=wv_d.ap())
                        if has_v_bias:
                            vb_sb2 = xvpool.tile([1, C], BF16)
                            nc.sync.dma_start(out=vb_sb2, in_=vb_d.ap())
                        else:
                            vb_sb2 = None
                        for hp in range(H // 2):
                            nc.scalar.dma_start(
                                out=wqk_sb[:, 2 * hp:2 * hp + 2],
                                in_=wqk_d.ap()[:, 2 * hp:2 * hp + 2])
                        for g in range(3):
                            nc.gpsimd.dma_start(
                                out=expb_sb[:, 2 * g:2 * g + 2, :],
                                in_=expb_d.ap()[:, 2 * g:2 * g + 2, :])
                        nc.sync.dma_start(out=wp_sb, in_=wp_d.ap())

                        # PE warm-up while DMAs land: keeps HAM at 2.4 GHz
                        if WARMUP:
                            pw = ps_qk.tile([128, QK_TW], F32, name="psqk", tag="psqk")
                            for w in range(WARMUP):
                                nc.tensor.matmul(
                                    pw, lhsT=junk_sb[:, 0:128], rhs=junk_sb[:, 0:QK_TW],
                                    start=(w == 0), stop=(w == WARMUP - 1),
                                )

                        # Pace the pipeline: the V matmuls and first attention
                        # pairs share the early blocks so per-block PE load is
                        # even. V blocks 0-1 first: their inputs (xT, wv) land
                        # before wqk, so the PE has work during the DMA ramp.
                        v_block(0, wv_sb, vb_sb2)
                        v_block(1, wv_sb, vb_sb2)
                        qk_mtile(0)
                        qk_mtile(1)
                        for hp in range(H // 2):
                            # PE filler between pair groups: the PE queue is
                            # FIFO, so dense matmul work must be emitted
                            # between attention pairs or the PE idles waiting
                            # on the softmax chain.
                            if hp + 1 < H // 2:
                                filler = [(2 * (hp + 1), nt) for nt in range(QK_NT)]
                                filler += [(2 * (hp + 1) + 1, nt) for nt in range(QK_NT)]
                            else:
                                filler = []
                            fi = [0]

                            def run_filler():
                                if hp == 0 and fb[0] + 2 < B_LOC:
                                    v_block(fb[0] + 2, wv_sb, vb_sb2)
                                    v_block(fb[0] + 3, wv_sb, vb_sb2)
                                    fb[0] += 2
                                for _ in range(2):
                                    if fi[0] < len(filler):
                                        qk_piece(*filler[fi[0]])
                                        fi[0] += 1

                            fb = [0]
                            for b in range(0, B_LOC, 2):
                                do_group(hp, b, run_filler)
                            while fi[0] < len(filler):
                                qk_piece(*filler[fi[0]])
                                fi[0] += 1
                        while len(pend) >= 2:
                            flush_av()
                    while inflight:
                        flush_back()

    nc.compile()
    return nc


def _bf16(a):
    return np.ascontiguousarray(a).astype(ml_dtypes.bfloat16)


def _host_prep(qkv_w, proj_w, rel_pos_table, rel_pos_index):
    """Shared (core-independent) SBUF images."""
    # exp of the transposed per-head bias, laid out as the kernel's score
    # tiles, two heads (one pair) side by side: head hh of pair hp occupies
    # columns hh*2N + [0, 2N): j-chunk0 rows 0:128, j-chunk1 rows 0:69 with
    # rows 69:128 zeroed (kills padded key rows).
    bias = rel_pos_table[rel_pos_index.reshape(-1)].reshape(N, N, H)  # [i, j, h]
    expb = np.zeros((128, H // 2, 4 * N), dtype=np.float32)
    eb = np.exp(bias)
    for h in range(H):
        ebT = eb[:, :, h].T  # [j, i]
        base = (h % 2) * 2 * N
        expb[:, h // 2, base:base + N] = ebT[0:128, :]
        expb[0:N - 128, h // 2, base + N:base + 2 * N] = ebT[128:N, :]
    expb16 = _bf16(expb)

    # wqk image [p, mt, kt, c], m-tile order [Q0,K0,Q1,K1,...]
    wqkT = np.ascontiguousarray(qkv_w[0:2 * C].T)            # [768, 1536]
    img = wqkT.reshape(KT, 128, 2, KT, 128).transpose(1, 3, 2, 0, 4)  # p,hp,s,kt,c
    wqk16 = _bf16(img.reshape(128, 12, KT, 128))

    wv16 = _bf16(qkv_w[2 * C:3 * C].T.reshape(KT, 128, C).transpose(1, 0, 2))
    wp16 = _bf16(proj_w.T.reshape(KT, 128, C).transpose(1, 0, 2))
    return wqk16, wv16, wp16, expb16


def _core_in_map(x, shared, core, has_q, has_v, has_p,
                 q_bias=None, v_bias=None, proj_b=None):
    wqk16, wv16, wp16, expb16 = shared
    xs = x[core * B_LOC:(core + 1) * B_LOC]                  # [8, 197, 768]
    # chunk-major image [p, chunk, kt, t'] so each chunk DMA is contiguous
    xT = _bf16(xs.reshape(NTOK, C).T.reshape(KT, 128, QK_NT, QK_TW)
               .transpose(1, 2, 0, 3))
    m = {"xT": xT, "wqk": wqk16, "wv": wv16, "wp": wp16, "expb": expb16}
    if has_q:
        m["qb"] = np.ascontiguousarray(q_bias.reshape(KT, 128).T)
    if has_v:
        m["vb"] = _bf16(v_bias.reshape(1, C))
    if has_p:
        m["pb"] = _bf16(proj_b.reshape(1, C))
    return m


def kernel(x, qkv_w, q_bias, v_bias, rel_pos_table, proj_w, proj_b, rel_pos_index):
    x = np.asarray(x, dtype=np.float32)
    qkv_w = np.asarray(qkv_w, dtype=np.float32)
    q_bias = np.asarray(q_bias, dtype=np.float32)
    v_bias = np.asarray(v_bias, dtype=np.float32)
    rel_pos_table = np.asarray(rel_pos_table, dtype=np.float32)
    proj_w = np.asarray(proj_w, dtype=np.float32)
    proj_b = np.asarray(proj_b, dtype=np.float32)
    rel_pos_index = np.asarray(rel_pos_index)

    has_q = bool(np.any(q_bias != 0))
    has_v = bool(np.any(v_bias != 0))
    has_p = bool(np.any(proj_b != 0))

    key = (has_v, has_p, has_q)
    if key not in _CACHE:
        _CACHE[key] = _build(*key)
    nc = _CACHE[key]

    shared = _host_prep(qkv_w, proj_w, rel_pos_table, rel_pos_index)
    in_maps = [
        _core_in_map(x, shared, c, has_q, has_v, has_p, q_bias, v_bias, proj_b)
        for c in range(N_CORES)
    ]

    res = bass_utils.run_bass_kernel_spmd(nc, in_maps, core_ids=list(range(N_CORES)))
    out = np.empty((B, N, C), dtype=np.float32)
    for c in range(N_CORES):
        out[c * B_LOC:(c + 1) * B_LOC] = np.asarray(
            res.results[c]["out"], dtype=np.float32).reshape(B_LOC, N, C)
    return out



# revision 8
# speedup vs baseline: 1.0813x; 1.0813x over previous
"""Trainium2 Bass kernel for ViT-style attention with relative position bias.

Module (per batch b, head h):
    qkv = x @ qkv_w.T + cat(q_bias, 0, v_bias)
    attn = softmax(scale * q @ k.T + bias[h])          bias = rel_pos_table[rel_pos_index]
    out  = (attn @ v) @ proj_w.T + proj_b

Distribution: pure data-parallel over batch — 8 NeuronCores x 8 batches each,
no collectives. Each core runs an identical SPMD program on its batch shard.

Device-side layout strategy (per core, all intermediates SBUF-resident bf16):
  - Host packs every input as its exact SBUF image so each tensor loads with
    a few large fully-contiguous DMAs, spread over four queues (sync: xT,
    scalar: wqk, vector: wv+wp, gpsimd: expb) with the first-needed chunks
    split in half so compute can start earlier.
  - While the first DMAs are in flight the PE runs warm-up matmuls on a junk
    tile so the HAM clock gate is at 2.4 GHz when real work starts.
  - qk^T matmul produces Q^T/K^T feature-major [64, tokens] slices directly,
    m-tile order interleaved [Q0,K0,Q1,K1,...].
  - V is token-major [tokens, 64] (the AV contraction needs keys on the
    partition axis), with a ones column block per head so the AV matmul also
    emits the softmax denominator rows for free.
  - Scores are computed transposed, S^T[j, i] = K[j] . Q[i]; softmax uses
    exp(s) * exp(bias) (no max-subtraction: logits are O(3), fp32/bf16 safe);
    exp(bias) is an input-derived constant computed host-side.
  - AV^T [64+64, 197] = [V_h | ones]^T @ expS^T is feature-major, feeding the
    proj matmul without transposes; the replicated denominator lands in
    partitions 64:128 and is reciprocal'd straight out of PSUM.
  - Schedule is batch-pair-group-outer: for each 394-token group (2 batches)
    all 12 heads' scores/AV run while the NEXT group's qk^T/V matmuls fill PE
    slack between attention pairs, and the PREVIOUS group's proj m-tiles
    stream out as soon as its last head is normalized. This spreads proj
    uniformly instead of bunching it in a drain phase at the end.
  - proj runs nt-outer so its two PSUM tiles retire alternately and the
    2-deep psum ring never head-blocks the PE.
  - Elementwise work is pinned per-engine (EXP: scalar, expb-mul mostly
    gpsimd, recip+normalize: vector, evacs balanced).
"""

import os
import numpy as np
import ml_dtypes

import concourse.bass as bass
import concourse.bacc as bacc
import concourse.mybir as mybir
import concourse.tile as tile
from concourse import bass_utils

F32 = mybir.dt.float32
BF16 = mybir.dt.bfloat16

N_CORES = 8
B = 64
B_LOC = B // N_CORES          # 8 batches per core
N = 197                       # tokens per batch
C = 768
H = 12
HD = 64
SCALE = HD ** -0.5
NTOK = B_LOC * N              # 1576
NPAD = 1664                   # 13 * 128 (qkT/aoT column allocation)
KT = 6                        # 768 / 128 contraction tiles
QK_NT = 4                     # n-tiles over tokens (= batch-pair groups)
QK_TW = NTOK // QK_NT         # 394

_CACHE = {}
WARMUP = int(os.environ.get("K_WARMUP", "16"))
OUTF32 = os.environ.get("K_OUTF32", "0") == "1"

# proj m-tile mt is ready once group (t_end-1)//QK_TW has all heads done
_PROJ_OF_GROUP = [[] for _ in range(QK_NT)]
for _mt in range(NPAD // 128):
    _t_end = min((_mt + 1) * 128, NTOK)
    _PROJ_OF_GROUP[(_t_end - 1) // QK_TW].append(_mt)


def _build(has_v_bias, has_p_bias, has_q_bias):
    nc = bacc.Bacc("TRN2", target_bir_lowering=False, debug=False)

    xT_d = nc.dram_tensor("xT", [128, QK_NT, KT, QK_TW], BF16, kind="ExternalInput")
    wqk_d = nc.dram_tensor("wqk", [128, 12, KT, 128], BF16, kind="ExternalInput")
    wv_d = nc.dram_tensor("wv", [128, KT, C], BF16, kind="ExternalInput")
    wp_d = nc.dram_tensor("wp", [128, KT, C], BF16, kind="ExternalInput")
    expb_d = nc.dram_tensor("expb", [128, H // 2, 4 * N], BF16, kind="ExternalInput")
    if has_q_bias:
        qb_d = nc.dram_tensor("qb", [128, KT], F32, kind="ExternalInput")
    if has_v_bias:
        vb_d = nc.dram_tensor("vb", [1, C], BF16, kind="ExternalInput")
    if has_p_bias:
        pb_d = nc.dram_tensor("pb", [1, C], BF16, kind="ExternalInput")
    out_d = nc.dram_tensor("out", [NTOK, C], F32 if OUTF32 else BF16, kind="ExternalOutput")

    with tile.TileContext(nc) as tc:
        with (
            tc.tile_pool(name="singles", bufs=1) as singles,
            tc.tile_pool(name="expwork", bufs=4) as expwork,
            tc.tile_pool(name="normwork", bufs=3) as normwork,
            tc.tile_pool(name="outstage", bufs=3) as outstage,
        ):
            # ---- persistent SBUF tensors ----
            wp_sb = singles.tile([128, KT, C], BF16)
            expb_sb = singles.tile([128, H // 2, 4 * N], BF16)
            # m-tile order [Q0, K0, Q1, K1, ...] to match wqk DMA chunks
            qkT_sb = singles.tile([128, 12, NPAD], BF16)
            # per-head blocks [V_h 64 | ones 64]; the ones half makes each AV
            # matmul replicate the softmax denominator into PSUM partitions
            # 64:128 (lhsT = [V_h | ones], 128 contiguous columns)
            V_sb = singles.tile([128, 2 * B_LOC, H, 2 * HD], BF16)
            aoT_sb = singles.tile([128, KT, NPAD], BF16)      # attn-out^T, proj stationary
            junk_sb = singles.tile([128, 512], BF16)

            if has_v_bias or has_p_bias:
                ones_row = singles.tile([1, NPAD], BF16)
            if has_p_bias:
                pb_sb = singles.tile([1, C], BF16)

            with tc.tile_pool(name="wqkpool", bufs=1) as wqkpool:
                wqk_sb = wqkpool.tile([128, 12, KT, 128], BF16)
                xT_sb = wqkpool.tile([128, QK_NT, KT, QK_TW], BF16)

                with (
                    tc.tile_pool(name="ps_qk", bufs=2, space="PSUM") as ps_qk,
                    tc.tile_pool(name="ps_s", bufs=1, space="PSUM") as ps_s_pool,
                    tc.tile_pool(name="ps_av2", bufs=2, space="PSUM") as ps_av_pool,
                    tc.tile_pool(name="xvpool", bufs=1) as xvpool,
                ):
                    wv_sb = xvpool.tile([128, KT, C], BF16)
                    if has_v_bias:
                        vb_sb2 = xvpool.tile([1, C], BF16)
                    else:
                        vb_sb2 = None

                    # ---- engine-ordered preamble ----------------------------
                    # Each engine's FIFO starts with its DMA enqueues so the
                    # transfers begin right after kernel boot; memsets that
                    # gate later stages follow, split across engines.
                    nc.vector.memset(junk_sb, 1.0)           # warm-up dep
                    # sync queue: xT chunk 0 split in half, then chunks 1-3
                    nc.sync.dma_start(out=xT_sb[:, 0, 0:3], in_=xT_d.ap()[:, 0, 0:3])
                    nc.sync.dma_start(out=xT_sb[:, 0, 3:6], in_=xT_d.ap()[:, 0, 3:6])
                    # scalar queue: wqk pair 0 split in half, then pairs 1-5
                    nc.scalar.dma_start(out=wqk_sb[:, 0:2, 0:3], in_=wqk_d.ap()[:, 0:2, 0:3])
                    nc.scalar.dma_start(out=wqk_sb[:, 0:2, 3:6], in_=wqk_d.ap()[:, 0:2, 3:6])
                    for hp in range(1, H // 2):
                        nc.scalar.dma_start(
                            out=wqk_sb[:, 2 * hp:2 * hp + 2],
                            in_=wqk_d.ap()[:, 2 * hp:2 * hp + 2])
                    # gpsimd queue: wv column-halves interleaved with expb pairs
                    # (V's nt=0 matmuls only need wv cols 0:384; expb pair hp
                    # is first needed at that pair's softmax multiply)
                    nc.gpsimd.dma_start(out=wv_sb[:, :, 0:C // 2],
                                        in_=wv_d.ap()[:, :, 0:C // 2])
                    nc.gpsimd.dma_start(
                        out=expb_sb[:, 0:1], in_=expb_d.ap()[:, 0:1])
                    nc.gpsimd.dma_start(out=wv_sb[:, :, C // 2:C],
                                        in_=wv_d.ap()[:, :, C // 2:C])
                    if has_v_bias:
                        nc.gpsimd.dma_start(out=vb_sb2, in_=vb_d.ap())
                    if has_q_bias:
                        qb_sb = singles.tile([128, KT], F32)
                        nc.sync.dma_start(out=qb_sb, in_=qb_d.ap())
                    if has_p_bias:
                        nc.gpsimd.dma_start(out=pb_sb, in_=pb_d.ap())
                    for hp in range(1, H // 2):
                        nc.gpsimd.dma_start(
                            out=expb_sb[:, hp:hp + 1], in_=expb_d.ap()[:, hp:hp + 1])
                    # sync queue: remaining xT chunks, then wp (first proj is
                    # not until the first group has all heads normalized)
                    for c in range(1, QK_NT):
                        nc.sync.dma_start(out=xT_sb[:, c], in_=xT_d.ap()[:, c])
                    nc.sync.dma_start(out=wp_sb, in_=wp_d.ap())

                    # V ones blocks: 4D-AP memsets, split across vector/gpsimd
                    if has_v_bias or has_p_bias:
                        nc.vector.memset(ones_row, 1.0)
                    nc.vector.memset(V_sb[:, 0:4, :, HD:2 * HD], 1.0)
                    nc.gpsimd.memset(V_sb[:, 4:8, :, HD:2 * HD], 1.0)
                    nc.vector.memset(V_sb[:, 8:12, :, HD:2 * HD], 1.0)
                    nc.gpsimd.memset(V_sb[:, 12:16, :, HD:2 * HD], 1.0)
                    # proj reads padded token columns of attn-out^T
                    nc.vector.memset(aoT_sb[:, :, NTOK:NPAD], 0.0)
                    # score lhsT for the last batch reads K-mtile cols past NTOK,
                    # and odd batches read a 59-col overhang into the next
                    # group's columns (garbage rows, masked by expb zero rows)
                    # — both must be initialized before first read.
                    for m in range(12):
                        if m % 2 == 1:
                            nc.gpsimd.memset(qkT_sb[:, m, NTOK:NPAD], 0.0)
                            for g in range(1, QK_NT):
                                nc.gpsimd.memset(
                                    qkT_sb[:, m, g * QK_TW:g * QK_TW + 2 * 128 - N], 0.0)

                    evac_n = [0]
                    # gpsimd cannot access PSUM — evacs go scalar/vector only
                    evac_engines = [nc.scalar, nc.vector]

                    def evac_copy(out, in_):
                        e = evac_engines[evac_n[0] % len(evac_engines)]
                        evac_n[0] += 1
                        if e is nc.scalar:
                            e.copy(out=out, in_=in_)
                        else:
                            e.tensor_copy(out=out, in_=in_)

                    def qk_piece(mt, nt):
                        pq = ps_qk.tile([128, QK_TW], F32, name="psqk", tag="psqk")
                        for kt in range(KT):
                            nc.tensor.matmul(
                                pq,
                                lhsT=wqk_sb[:, mt, kt, :],
                                rhs=xT_sb[:, nt, kt, :],
                                start=(kt == 0), stop=(kt == KT - 1),
                            )
                        dst = qkT_sb[:, mt, nt * QK_TW:(nt + 1) * QK_TW]
                        if has_q_bias and mt % 2 == 0:
                            nc.any.tensor_scalar_add(dst, pq, qb_sb[:, mt // 2:mt // 2 + 1])
                        else:
                            nc.vector.tensor_copy(out=dst, in_=pq)

                    def v_block(b):
                        # token-major V for one batch, psum borrowed from ps_qk
                        for jc in range(2):
                            m = 128 if jc == 0 else N - 128
                            tok0 = b * N + jc * 128
                            tc_, to_ = tok0 // QK_TW, tok0 % QK_TW
                            for nt in range(2):
                                pv = ps_qk.tile([128, QK_TW], F32, name="psqk", tag="psqk")
                                for kt in range(KT):
                                    nc.tensor.matmul(
                                        pv[0:m, 0:C // 2],
                                        lhsT=xT_sb[:, tc_, kt, to_:to_ + m],
                                        rhs=wv_sb[:, kt, nt * (C // 2):(nt + 1) * (C // 2)],
                                        start=(kt == 0),
                                        stop=(kt == KT - 1 and not has_v_bias),
                                    )
                                if has_v_bias:
                                    nc.tensor.matmul(
                                        pv[0:m, 0:C // 2],
                                        lhsT=ones_row[:, tok0:tok0 + m],
                                        rhs=vb_sb2[:, nt * (C // 2):(nt + 1) * (C // 2)],
                                        start=False, stop=True,
                                    )
                                evac_copy(
                                    V_sb[0:m, b * 2 + jc, nt * 6:(nt + 1) * 6, 0:HD],
                                    pv[0:m, 0:C // 2].rearrange("p (g f) -> p g f", g=KT),
                                )

                    def attn_scores(hp, b, mul_eng):
                        q0 = b * N
                        # Both heads' scores in one 2-bank PSUM tile (head hh at
                        # columns hh*512 + [0, 394)) so one bank-hopping AP
                        # covers the pair in a single EXP. Even/odd heads live
                        # at base partitions 0/64, so their k=64 score matmuls
                        # occupy disjoint PE row groups and overlap.
                        ps_s2 = ps_s_pool.tile([128, 1024], F32, name="ps_s2", tag="ps_s2")
                        for jc in range(2):
                            for hh in range(2):
                                h = hp * 2 + hh
                                pbase = (h % 2) * 64
                                nc.tensor.matmul(
                                    ps_s2[:, hh * 512 + jc * N:hh * 512 + (jc + 1) * N],
                                    lhsT=qkT_sb[pbase:pbase + 64, 2 * hp + 1,
                                                q0 + jc * 128:q0 + jc * 128 + 128],
                                    rhs=qkT_sb[pbase:pbase + 64, 2 * hp, q0:q0 + N],
                                    start=(jc == 0), stop=(jc == 1),
                                )
                        expS2 = expwork.tile([128, 4 * N], BF16, tag="expS2")
                        nc.scalar.activation(
                            expS2.rearrange("p (g w) -> p g w", w=2 * N),
                            ps_s2.rearrange("p (g w) -> p g w", w=512)[:, :, 0:2 * N],
                            mybir.ActivationFunctionType.Exp, scale=SCALE,
                        )
                        expST2 = expwork.tile([128, 4 * N], BF16, tag="expST2")
                        mul_eng.tensor_mul(expST2, expS2, expb_sb[:, hp, :])
                        return expST2

                    def attn_av(hp, b, av, expST2):
                        for hh in range(2):
                            h = hp * 2 + hh
                            for jc in range(2):
                                jm = 128 if jc == 0 else N - 128
                                nc.tensor.matmul(
                                    av[:, hh * N:(hh + 1) * N],
                                    lhsT=V_sb[0:jm, b * 2 + jc, h, :],
                                    rhs=expST2[0:jm, hh * 2 * N + jc * N:hh * 2 * N + (jc + 1) * N],
                                    start=(hh == 0 and jc == 0),
                                    stop=(hh == 1 and jc == 1),
                                )

                    def attn_back2(hp, b, av2):
                        # one copy/recip/mul set covers pairs (hp,b) and (hp,b+1)
                        denom2 = normwork.tile([64, 2, 2 * N], F32, tag="denom")
                        nc.scalar.copy(out=denom2, in_=av2[64:128, :, 0:2 * N])
                        recipB2 = normwork.tile([64, 2, 2 * N], F32, tag="recipB")
                        nc.vector.reciprocal_approx_fast(recipB2, denom2)
                        for hh in range(2):
                            h = hp * 2 + hh
                            dst = aoT_sb[(h % 2) * 64:(h % 2) * 64 + 64, h // 2,
                                         b * N:(b + 2) * N]
                            nc.vector.tensor_mul(
                                dst.rearrange("p (n w) -> p n w", n=2),
                                av2[0:HD, :, hh * N:(hh + 1) * N],
                                recipB2[:, :, hh * N:(hh + 1) * N],
                            )

                    out_dma = [nc.sync, nc.scalar]

                    def proj_mtile(mt):
                        rows = min(128, NTOK - mt * 128)
                        stage = outstage.tile([128, C], F32 if OUTF32 else BF16, tag="stage")
                        # nt-outer: the two psum tiles retire alternately so the
                        # 2-deep ring never stalls the PE between m-tiles
                        for nt in range(2):
                            ps = ps_qk.tile([128, QK_TW], F32, name="psqk", tag="psqk")
                            for kt in range(KT):
                                nc.tensor.matmul(
                                    ps[:, 0:C // 2],
                                    lhsT=aoT_sb[:, kt, mt * 128:(mt + 1) * 128],
                                    rhs=wp_sb[:, kt, nt * (C // 2):(nt + 1) * (C // 2)],
                                    start=(kt == 0),
                                    stop=(kt == KT - 1 and not has_p_bias),
                                )
                            if has_p_bias:
                                nc.tensor.matmul(
                                    ps[:, 0:C // 2],
                                    lhsT=ones_row[:, mt * 128:(mt + 1) * 128],
                                    rhs=pb_sb[:, nt * (C // 2):(nt + 1) * (C // 2)],
                                    start=False, stop=True,
                                )
                            if nt == 0:
                                nc.scalar.copy(out=stage[:, 0:C // 2], in_=ps[:, 0:C // 2])
                            else:
                                nc.vector.tensor_copy(out=stage[:, C // 2:C], in_=ps[:, 0:C // 2])
                        out_dma[0 if OUTF32 else mt % 2].dma_start(
                            out=out_d.ap()[mt * 128:mt * 128 + rows, :], in_=stage[0:rows, :]
                        )

                    # ---- filler unit scheduler ------------------------------
                    # Units are dense work (qk pieces, V blocks, proj m-tiles,
                    # deferred memsets) emitted between attention pairs so the
                    # FIFO PE queue never head-blocks on the softmax chain.
                    units = []
                    qk_done = set()

                    def qk_unit(mt, nt):
                        return ("qk", (mt, nt), lambda: qk_piece(mt, nt))

                    def v_unit(b):
                        return ("v", b, lambda: v_block(b))

                    def proj_unit(mt):
                        return ("proj", mt, lambda: proj_mtile(mt))

                    def run_unit(u):
                        if u[0] == "qk":
                            qk_done.add(u[1])
                        u[2]()

                    def filler_step(n=2):
                        for _ in range(n):
                            if units:
                                run_unit(units.pop(0))

                    def ensure_qk(mt, nt):
                        while (mt, nt) not in qk_done and units:
                            run_unit(units.pop(0))

                    inflight = []   # av2 groups awaiting back2
                    pend = []       # scored pairs awaiting their AV matmuls
                    pair_n = [0]
                    # 3:1 gpsimd:vector for the exp(bias) multiply (SBUF-only op)
                    mul_cycle = [nc.gpsimd, nc.gpsimd, nc.vector, nc.gpsimd]

                    def flush_av():
                        # emit AV matmuls for the two OLDEST scored pairs —
                        # their expb-muls got a full slot of slack, so the
                        # FIFO PE queue never head-blocks on the softmax chain
                        (h0, b0, e0), (h1, b1, e1) = pend.pop(0), pend.pop(0)
                        av2 = ps_av_pool.tile([128, 2, 512], F32, name="av2", tag="av2")
                        attn_av(h0, b0, av2[:, 0, 0:2 * N], e0)
                        attn_av(h1, b1, av2[:, 1, 0:2 * N], e1)
                        inflight.append((h0, b0, av2))

                    def flush_back():
                        hp, b, av2 = inflight.pop(0)
                        attn_back2(hp, b, av2)
                        if hp == H // 2 - 1:
                            units.extend(proj_unit(mt) for mt in _PROJ_OF_GROUP[b // 2])

                    # PE warm-up while DMAs land: keeps HAM at 2.4 GHz
                    if WARMUP:
                        pw = ps_qk.tile([128, QK_TW], F32, name="psqk", tag="psqk")
                        for w in range(WARMUP):
                            nc.tensor.matmul(
                                pw, lhsT=junk_sb[:, 0:128], rhs=junk_sb[:, 0:QK_TW],
                                start=(w == 0), stop=(w == WARMUP - 1),
                            )

                    # ---- prologue: group 0's first head pairs + V ----------
                    for mt in range(4):
                        qk_piece(mt, 0)
                        qk_done.add((mt, 0))
                    v_block(0)
                    v_block(1)
                    units.extend(qk_unit(mt, 0) for mt in range(4, 12))

                    # ---- main loop: batch-pair groups ----------------------
                    for g in range(QK_NT):
                        if g + 1 < QK_NT:
                            nxt = [qk_unit(mt, g + 1) for mt in range(4)]
                            nxt.append(v_unit(2 * (g + 1)))
                            nxt.append(v_unit(2 * (g + 1) + 1))
                            nxt += [qk_unit(mt, g + 1) for mt in range(4, 12)]
                            units.extend(nxt)
                        for hp in range(H // 2):
                            ensure_qk(2 * hp, g)
                            ensure_qk(2 * hp + 1, g)
                            for db in range(2):
                                mul_eng = mul_cycle[pair_n[0] % len(mul_cycle)]
                                pair_n[0] += 1
                                pend.append((hp, 2 * g + db,
                                             attn_scores(hp, 2 * g + db, mul_eng)))
                            if len(inflight) >= 2:
                                flush_back()
                            filler_step(2)
                            if len(pend) >= 4:
                                flush_av()

                    # ---- drain --------------------------------------------
                    while len(pend) >= 2:
                        flush_av()
                        if len(inflight) >= 2:
                            flush_back()
                            filler_step(1)
                    while inflight:
                        flush_back()
                        filler_step(2)
                    while units:
                        run_unit(units.pop(0))

    nc.compile()
    return nc


def _bf16(a):
    return np.ascontiguousarray(a).astype(ml_dtypes.bfloat16)


def _host_prep(qkv_w, proj_w, rel_pos_table, rel_pos_index):
    """Shared (core-independent) SBUF images."""
    # exp of the transposed per-head bias, laid out as the kernel's score
    # tiles, two heads (one pair) side by side: head hh of pair hp occupies
    # columns hh*2N + [0, 2N): j-chunk0 rows 0:128, j-chunk1 rows 0:69 with
    # rows 69:128 zeroed (kills padded key rows).
    bias = rel_pos_table[rel_pos_index.reshape(-1)].reshape(N, N, H)  # [i, j, h]
    expb = np.zeros((128, H // 2, 4 * N), dtype=np.float32)
    eb = np.exp(bias)
    for h in range(H):
        ebT = eb[:, :, h].T  # [j, i]
        base = (h % 2) * 2 * N
        expb[:, h // 2, base:base + N] = ebT[0:128, :]
        expb[0:N - 128, h // 2, base + N:base + 2 * N] = ebT[128:N, :]
    expb16 = _bf16(expb)

    # wqk image [p, mt, kt, c], m-tile order [Q0,K0,Q1,K1,...]
    wqkT = np.ascontiguousarray(qkv_w[0:2 * C].T)            # [768, 1536]
    img = wqkT.reshape(KT, 128, 2, KT, 128).transpose(1, 3, 2, 0, 4)  # p,hp,s,kt,c
    wqk16 = _bf16(img.reshape(128, 12, KT, 128))

    wv16 = _bf16(qkv_w[2 * C:3 * C].T.reshape(KT, 128, C).transpose(1, 0, 2))
    wp16 = _bf16(proj_w.T.reshape(KT, 128, C).transpose(1, 0, 2))
    return wqk16, wv16, wp16, expb16


def _core_in_map(x, shared, core, has_q, has_v, has_p,
                 q_bias=None, v_bias=None, proj_b=None):
    wqk16, wv16, wp16, expb16 = shared
    xs = x[core * B_LOC:(core + 1) * B_LOC]                  # [8, 197, 768]
    # chunk-major image [p, chunk, kt, t'] so each chunk DMA is contiguous
    xT = _bf16(xs.reshape(NTOK, C).T.reshape(KT, 128, QK_NT, QK_TW)
               .transpose(1, 2, 0, 3))
    m = {"xT": xT, "wqk": wqk16, "wv": wv16, "wp": wp16, "expb": expb16}
    if has_q:
        m["qb"] = np.ascontiguousarray(q_bias.reshape(KT, 128).T)
    if has_v:
        m["vb"] = _bf16(v_bias.reshape(1, C))
    if has_p:
        m["pb"] = _bf16(proj_b.reshape(1, C))
    return m


def kernel(x, qkv_w, q_bias, v_bias, rel_pos_table, proj_w, proj_b, rel_pos_index):
    x = np.asarray(x, dtype=np.float32)
    qkv_w = np.asarray(qkv_w, dtype=np.float32)
    q_bias = np.asarray(q_bias, dtype=np.float32)
    v_bias = np.asarray(v_bias, dtype=np.float32)
    rel_pos_table = np.asarray(rel_pos_table, dtype=np.float32)
    proj_w = np.asarray(proj_w, dtype=np.float32)
    proj_b = np.asarray(proj_b, dtype=np.float32)
    rel_pos_index = np.asarray(rel_pos_index)

    has_q = bool(np.any(q_bias != 0))
    has_v = bool(np.any(v_bias != 0))
    has_p = bool(np.any(proj_b != 0))

    key = (has_v, has_p, has_q)
    if key not in _CACHE:
        _CACHE[key] = _build(*key)
    nc = _CACHE[key]

    shared = _host_prep(qkv_w, proj_w, rel_pos_table, rel_pos_index)
    in_maps = [
        _core_in_map(x, shared, c, has_q, has_v, has_p, q_bias, v_bias, proj_b)
        for c in range(N_CORES)
    ]

    res = bass_utils.run_bass_kernel_spmd(nc, in_maps, core_ids=list(range(N_CORES)))
    out = np.empty((B, N, C), dtype=np.float32)
    for c in range(N_CORES):
        out[c * B_LOC:(c + 1) * B_LOC] = np.asarray(
            res.results[c]["out"], dtype=np.float32).reshape(B_LOC, N, C)
    return out


# revision 13
# speedup vs baseline: 1.0835x; 1.0021x over previous
"""Trainium2 Bass kernel for ViT-style attention with relative position bias.

Module (per batch b, head h):
    qkv = x @ qkv_w.T + cat(q_bias, 0, v_bias)
    attn = softmax(scale * q @ k.T + bias[h])          bias = rel_pos_table[rel_pos_index]
    out  = (attn @ v) @ proj_w.T + proj_b

Distribution: pure data-parallel over batch — 8 NeuronCores x 8 batches each,
no collectives. Each core runs an identical SPMD program on its batch shard.

Device-side layout strategy (per core, all intermediates SBUF-resident bf16):
  - Host packs every input as its exact SBUF image so each tensor loads with
    a few large fully-contiguous DMAs, spread over four queues (sync: xT,
    scalar: wqk, vector: wv+wp, gpsimd: expb) with the first-needed chunks
    split in half so compute can start earlier.
  - While the first DMAs are in flight the PE runs warm-up matmuls on a junk
    tile so the HAM clock gate is at 2.4 GHz when real work starts.
  - qk^T matmul produces Q^T/K^T feature-major [64, tokens] slices directly,
    m-tile order interleaved [Q0,K0,Q1,K1,...].
  - V is token-major [tokens, 64] (the AV contraction needs keys on the
    partition axis), with a ones column block per head so the AV matmul also
    emits the softmax denominator rows for free.
  - Scores are computed transposed, S^T[j, i] = K[j] . Q[i]; softmax uses
    exp(s) * exp(bias) (no max-subtraction: logits are O(3), fp32/bf16 safe);
    exp(bias) is an input-derived constant computed host-side.
  - AV^T [64+64, 197] = [V_h | ones]^T @ expS^T is feature-major, feeding the
    proj matmul without transposes; the replicated denominator lands in
    partitions 64:128 and is reciprocal'd straight out of PSUM.
  - Schedule is batch-pair-group-outer: for each 394-token group (2 batches)
    all 12 heads' scores/AV run while the NEXT group's qk^T/V matmuls fill PE
    slack between attention pairs, and the PREVIOUS group's proj m-tiles
    stream out as soon as its last head is normalized. This spreads proj
    uniformly instead of bunching it in a drain phase at the end.
  - proj runs nt-outer so its two PSUM tiles retire alternately and the
    2-deep psum ring never head-blocks the PE.
  - Elementwise work is pinned per-engine (EXP: scalar, expb-mul mostly
    gpsimd, recip+normalize: vector, evacs balanced).
"""

import os
import numpy as np
import ml_dtypes

import concourse.bass as bass
import concourse.bacc as bacc
import concourse.mybir as mybir
import concourse.tile as tile
from concourse import bass_utils

F32 = mybir.dt.float32
BF16 = mybir.dt.bfloat16

N_CORES = 8
B = 64
B_LOC = B // N_CORES          # 8 batches per core
N = 197                       # tokens per batch
C = 768
H = 12
HD = 64
SCALE = HD ** -0.5
NTOK = B_LOC * N              # 1576
NPAD = 1664                   # 13 * 128 (qkT/aoT column allocation)
KT = 6                        # 768 / 128 contraction tiles
QK_NT = 4                     # n-tiles over tokens (= batch-pair groups)
QK_TW = NTOK // QK_NT         # 394

_CACHE = {}
WARMUP = int(os.environ.get("K_WARMUP", "20"))
OUTF32 = os.environ.get("K_OUTF32", "0") == "1"

# proj m-tile mt is ready once group (t_end-1)//QK_TW has all heads done
_PROJ_OF_GROUP = [[] for _ in range(QK_NT)]
for _mt in range(NPAD // 128):
    _t_end = min((_mt + 1) * 128, NTOK)
    _PROJ_OF_GROUP[(_t_end - 1) // QK_TW].append(_mt)


def _build(has_v_bias, has_p_bias, has_q_bias):
    nc = bacc.Bacc("TRN2", target_bir_lowering=False, debug=False)

    xT_d = nc.dram_tensor("xT", [128, QK_NT, KT, QK_TW], BF16, kind="ExternalInput")
    wqk_d = nc.dram_tensor("wqk", [128, 12, KT, 128], BF16, kind="ExternalInput")
    wv_d = nc.dram_tensor("wv", [128, KT, C], BF16, kind="ExternalInput")
    wp_d = nc.dram_tensor("wp", [128, KT, C], BF16, kind="ExternalInput")
    expb_d = nc.dram_tensor("expb", [128, H // 2, 4 * N], BF16, kind="ExternalInput")
    if has_q_bias:
        qb_d = nc.dram_tensor("qb", [128, KT], F32, kind="ExternalInput")
    if has_v_bias:
        vb_d = nc.dram_tensor("vb", [1, C], BF16, kind="ExternalInput")
    if has_p_bias:
        pb_d = nc.dram_tensor("pb", [1, C], BF16, kind="ExternalInput")
    out_d = nc.dram_tensor("out", [NTOK, C], F32 if OUTF32 else BF16, kind="ExternalOutput")

    with tile.TileContext(nc) as tc:
        with (
            tc.tile_pool(name="singles", bufs=1) as singles,
            tc.tile_pool(name="expwork", bufs=4) as expwork,
            tc.tile_pool(name="normwork", bufs=3) as normwork,
            tc.tile_pool(name="outstage", bufs=3) as outstage,
        ):
            # ---- persistent SBUF tensors ----
            wp_sb = singles.tile([128, KT, C], BF16)
            expb_sb = singles.tile([128, H // 2, 4 * N], BF16)
            # m-tile order [Q0, K0, Q1, K1, ...] to match wqk DMA chunks
            qkT_sb = singles.tile([128, 12, NPAD], BF16)
            # per-head blocks [V_h 64 | ones 64]; the ones half makes each AV
            # matmul replicate the softmax denominator into PSUM partitions
            # 64:128 (lhsT = [V_h | ones], 128 contiguous columns)
            V_sb = singles.tile([128, 2 * B_LOC, H, 2 * HD], BF16)
            aoT_sb = singles.tile([128, KT, NPAD], BF16)      # attn-out^T, proj stationary
            junk_sb = singles.tile([128, 512], BF16)

            if has_v_bias or has_p_bias:
                ones_row = singles.tile([1, NPAD], BF16)
            if has_p_bias:
                pb_sb = singles.tile([1, C], BF16)

            with tc.tile_pool(name="wqkpool", bufs=1) as wqkpool:
                wqk_sb = wqkpool.tile([128, 12, KT, 128], BF16)
                xT_sb = wqkpool.tile([128, QK_NT, KT, QK_TW], BF16)

                with (
                    tc.tile_pool(name="ps_qk", bufs=2, space="PSUM") as ps_qk,
                    tc.tile_pool(name="ps_s", bufs=1, space="PSUM") as ps_s_pool,
                    tc.tile_pool(name="ps_av2", bufs=2, space="PSUM") as ps_av_pool,
                    tc.tile_pool(name="xvpool", bufs=1) as xvpool,
                ):
                    wv_sb = xvpool.tile([128, KT, C], BF16)
                    if has_v_bias:
                        vb_sb2 = xvpool.tile([1, C], BF16)
                    else:
                        vb_sb2 = None

                    # ---- engine-ordered preamble ----------------------------
                    # ~8 DMA transfers can be in flight (HWDGE+SWDGE lanes) and
                    # concurrent transfers fair-share HBM bandwidth, so bulk
                    # loads would starve the critical first-group tensors.
                    # Wave 1 (enqueued here) carries only what the first ~15us
                    # of compute needs; the rest is enqueued from the scalar
                    # engine's FIFO at later schedule points (see _gates), so
                    # its EXP pacing acts as a wall-clock gate.
                    nc.vector.memset(junk_sb, 1.0)           # warm-up dep
                    # sync queue: xT chunk 0 in two halves (smaller transfers
                    # finish earlier under fair-share)
                    nc.sync.dma_start(out=xT_sb[:, 0, 0:3], in_=xT_d.ap()[:, 0, 0:3])
                    nc.sync.dma_start(out=xT_sb[:, 0, 3:6], in_=xT_d.ap()[:, 0, 3:6])
                    # scalar queue: wqk pair 0 in two halves
                    nc.scalar.dma_start(out=wqk_sb[:, 0:2, 0:3], in_=wqk_d.ap()[:, 0:2, 0:3])
                    nc.scalar.dma_start(out=wqk_sb[:, 0:2, 3:6], in_=wqk_d.ap()[:, 0:2, 3:6])
                    # gpsimd queue: wv kt-halves (contiguous), first expb pairs,
                    # then xT chunk 1 (needed by the next group's qk/V filler)
                    nc.gpsimd.dma_start(out=wv_sb[:, 0:3], in_=wv_d.ap()[:, 0:3])
                    nc.gpsimd.dma_start(out=wv_sb[:, 3:6], in_=wv_d.ap()[:, 3:6])
                    nc.gpsimd.dma_start(out=expb_sb[:, 0:1], in_=expb_d.ap()[:, 0:1])
                    nc.gpsimd.dma_start(out=expb_sb[:, 1:2], in_=expb_d.ap()[:, 1:2])
                    nc.gpsimd.dma_start(out=xT_sb[:, 1, 0:3], in_=xT_d.ap()[:, 1, 0:3])
                    nc.gpsimd.dma_start(out=xT_sb[:, 1, 3:6], in_=xT_d.ap()[:, 1, 3:6])
                    if has_v_bias:
                        nc.gpsimd.dma_start(out=vb_sb2, in_=vb_d.ap())
                    if has_q_bias:
                        qb_sb = singles.tile([128, KT], F32)
                        nc.sync.dma_start(out=qb_sb, in_=qb_d.ap())
                    if has_p_bias:
                        nc.gpsimd.dma_start(out=pb_sb, in_=pb_d.ap())

                    # all memsets on vector: it has no early work (first qk
                    # evac is ~15us in) while gpsimd must be free for the
                    # first softmax multiplies
                    if has_v_bias or has_p_bias:
                        nc.vector.memset(ones_row, 1.0)
                    nc.vector.memset(V_sb[:, 0:8, :, HD:2 * HD], 1.0)
                    nc.vector.memset(V_sb[:, 8:16, :, HD:2 * HD], 1.0)
                    # proj reads padded token columns of attn-out^T
                    nc.vector.memset(aoT_sb[:, :, NTOK:NPAD], 0.0)
                    # score lhsT for the last batch reads K-mtile cols past NTOK,
                    # and odd batches read a 59-col overhang into the next
                    # group's columns (garbage rows, masked by expb zero rows)
                    # — both must be initialized before first read.
                    for m in range(12):
                        if m % 2 == 1:
                            nc.vector.memset(qkT_sb[:, m, NTOK:NPAD], 0.0)
                            for g in range(1, QK_NT):
                                nc.vector.memset(
                                    qkT_sb[:, m, g * QK_TW:g * QK_TW + 2 * 128 - N], 0.0)

                    # staged bulk DMA enqueues, fired from the scalar FIFO at
                    # (group, head-pair) points in the schedule
                    def _wqk_pair(hp):
                        return lambda: nc.scalar.dma_start(
                            out=wqk_sb[:, 2 * hp:2 * hp + 2],
                            in_=wqk_d.ap()[:, 2 * hp:2 * hp + 2])

                    def _expb_pair(hp):
                        return lambda: nc.scalar.dma_start(
                            out=expb_sb[:, hp:hp + 1], in_=expb_d.ap()[:, hp:hp + 1])

                    def _xT_half(c, h):
                        return lambda: nc.scalar.dma_start(
                            out=xT_sb[:, c, 3 * h:3 * h + 3],
                            in_=xT_d.ap()[:, c, 3 * h:3 * h + 3])

                    def _wp_half(h):
                        return lambda: nc.scalar.dma_start(
                            out=wp_sb[:, 3 * h:3 * h + 3], in_=wp_d.ap()[:, 3 * h:3 * h + 3])

                    _gates = {
                        (0, 0): [_wqk_pair(1), _wqk_pair(2)],
                        (0, 1): [_wqk_pair(3), _expb_pair(2)],
                        (0, 2): [_wqk_pair(4), _expb_pair(3)],
                        (0, 3): [_wqk_pair(5), _expb_pair(4)],
                        (0, 4): [_expb_pair(5), _xT_half(2, 0)],
                        (0, 5): [_xT_half(2, 1), _wp_half(0)],
                        (1, 0): [_wp_half(1), _xT_half(3, 0)],
                        (1, 1): [_xT_half(3, 1)],
                    }

                    evac_n = [0]
                    # gpsimd cannot access PSUM — evacs go scalar/vector only
                    evac_engines = [nc.scalar, nc.vector]

                    def evac_copy(out, in_):
                        e = evac_engines[evac_n[0] % len(evac_engines)]
                        evac_n[0] += 1
                        if e is nc.scalar:
                            e.copy(out=out, in_=in_)
                        else:
                            e.tensor_copy(out=out, in_=in_)

                    def qk_piece(mt, nt):
                        pq = ps_qk.tile([128, QK_TW], F32, name="psqk", tag="psqk")
                        for kt in range(KT):
                            nc.tensor.matmul(
                                pq,
                                lhsT=wqk_sb[:, mt, kt, :],
                                rhs=xT_sb[:, nt, kt, :],
                                start=(kt == 0), stop=(kt == KT - 1),
                            )
                        dst = qkT_sb[:, mt, nt * QK_TW:(nt + 1) * QK_TW]
                        if has_q_bias and mt % 2 == 0:
                            nc.any.tensor_scalar_add(dst, pq, qb_sb[:, mt // 2:mt // 2 + 1])
                        else:
                            nc.vector.tensor_copy(out=dst, in_=pq)

                    def v_block(b):
                        # token-major V for one batch, psum borrowed from ps_qk
                        for jc in range(2):
                            m = 128 if jc == 0 else N - 128
                            tok0 = b * N + jc * 128
                            tc_, to_ = tok0 // QK_TW, tok0 % QK_TW
                            for nt in range(2):
                                pv = ps_qk.tile([128, QK_TW], F32, name="psqk", tag="psqk")
                                for kt in range(KT):
                                    nc.tensor.matmul(
                                        pv[0:m, 0:C // 2],
                                        lhsT=xT_sb[:, tc_, kt, to_:to_ + m],
                                        rhs=wv_sb[:, kt, nt * (C // 2):(nt + 1) * (C // 2)],
                                        start=(kt == 0),
                                        stop=(kt == KT - 1 and not has_v_bias),
                                    )
                                if has_v_bias:
                                    nc.tensor.matmul(
                                        pv[0:m, 0:C // 2],
                                        lhsT=ones_row[:, tok0:tok0 + m],
                                        rhs=vb_sb2[:, nt * (C // 2):(nt + 1) * (C // 2)],
                                        start=False, stop=True,
                                    )
                                evac_copy(
                                    V_sb[0:m, b * 2 + jc, nt * 6:(nt + 1) * 6, 0:HD],
                                    pv[0:m, 0:C // 2].rearrange("p (g f) -> p g f", g=KT),
                                )

                    def attn_scores(hp, b, mul_eng):
                        q0 = b * N
                        # Both heads' scores in one 2-bank PSUM tile (head hh at
                        # columns hh*512 + [0, 394)) so one bank-hopping AP
                        # covers the pair in a single EXP. Even/odd heads live
                        # at base partitions 0/64, so their k=64 score matmuls
                        # occupy disjoint PE row groups and overlap.
                        ps_s2 = ps_s_pool.tile([128, 1024], F32, name="ps_s2", tag="ps_s2")
                        for jc in range(2):
                            for hh in range(2):
                                h = hp * 2 + hh
                                pbase = (h % 2) * 64
                                nc.tensor.matmul(
                                    ps_s2[:, hh * 512 + jc * N:hh * 512 + (jc + 1) * N],
                                    lhsT=qkT_sb[pbase:pbase + 64, 2 * hp + 1,
                                                q0 + jc * 128:q0 + jc * 128 + 128],
                                    rhs=qkT_sb[pbase:pbase + 64, 2 * hp, q0:q0 + N],
                                    start=(jc == 0), stop=(jc == 1),
                                )
                        expS2 = expwork.tile([128, 4 * N], BF16, tag="expS2")
                        nc.scalar.activation(
                            expS2.rearrange("p (g w) -> p g w", w=2 * N),
                            ps_s2.rearrange("p (g w) -> p g w", w=512)[:, :, 0:2 * N],
                            mybir.ActivationFunctionType.Exp, scale=SCALE,
                        )
                        expST2 = expwork.tile([128, 4 * N], BF16, tag="expST2")
                        mul_eng.tensor_mul(expST2, expS2, expb_sb[:, hp, :])
                        return expST2

                    def attn_av(hp, b, av, expST2):
                        for hh in range(2):
                            h = hp * 2 + hh
                            for jc in range(2):
                                jm = 128 if jc == 0 else N - 128
                                nc.tensor.matmul(
                                    av[:, hh * N:(hh + 1) * N],
                                    lhsT=V_sb[0:jm, b * 2 + jc, h, :],
                                    rhs=expST2[0:jm, hh * 2 * N + jc * N:hh * 2 * N + (jc + 1) * N],
                                    start=(hh == 0 and jc == 0),
                                    stop=(hh == 1 and jc == 1),
                                )

                    def attn_back2(hp, b, av2):
                        # one copy/recip/mul set covers pairs (hp,b) and (hp,b+1)
                        denom2 = normwork.tile([64, 2, 2 * N], F32, tag="denom")
                        nc.scalar.copy(out=denom2, in_=av2[64:128, :, 0:2 * N])
                        recipB2 = normwork.tile([64, 2, 2 * N], F32, tag="recipB")
                        nc.vector.reciprocal_approx_fast(recipB2, denom2)
                        for hh in range(2):
                            h = hp * 2 + hh
                            dst = aoT_sb[(h % 2) * 64:(h % 2) * 64 + 64, h // 2,
                                         b * N:(b + 2) * N]
                            nc.vector.tensor_mul(
                                dst.rearrange("p (n w) -> p n w", n=2),
                                av2[0:HD, :, hh * N:(hh + 1) * N],
                                recipB2[:, :, hh * N:(hh + 1) * N],
                            )

                    out_dma = [nc.sync, nc.scalar]

                    def proj_mtile(mt):
                        rows = min(128, NTOK - mt * 128)
                        stage = outstage.tile([128, C], F32 if OUTF32 else BF16, tag="stage")
                        # nt-outer: the two psum tiles retire alternately so the
                        # 2-deep ring never stalls the PE between m-tiles
                        for nt in range(2):
                            ps = ps_qk.tile([128, QK_TW], F32, name="psqk", tag="psqk")
                            for kt in range(KT):
                                nc.tensor.matmul(
                                    ps[:, 0:C // 2],
                                    lhsT=aoT_sb[:, kt, mt * 128:(mt + 1) * 128],
                                    rhs=wp_sb[:, kt, nt * (C // 2):(nt + 1) * (C // 2)],
                                    start=(kt == 0),
                                    stop=(kt == KT - 1 and not has_p_bias),
                                )
                            if has_p_bias:
                                nc.tensor.matmul(
                                    ps[:, 0:C // 2],
                                    lhsT=ones_row[:, mt * 128:(mt + 1) * 128],
                                    rhs=pb_sb[:, nt * (C // 2):(nt + 1) * (C // 2)],
                                    start=False, stop=True,
                                )
                            if nt == 0:
                                nc.scalar.copy(out=stage[:, 0:C // 2], in_=ps[:, 0:C // 2])
                            else:
                                nc.vector.tensor_copy(out=stage[:, C // 2:C], in_=ps[:, 0:C // 2])
                        out_dma[0 if OUTF32 else mt % 2].dma_start(
                            out=out_d.ap()[mt * 128:mt * 128 + rows, :], in_=stage[0:rows, :]
                        )

                    # ---- filler unit scheduler ------------------------------
                    # Units are dense work (qk pieces, V blocks, proj m-tiles,
                    # deferred memsets) emitted between attention pairs so the
                    # FIFO PE queue never head-blocks on the softmax chain.
                    units = []
                    qk_done = set()

                    def qk_unit(mt, nt):
                        return ("qk", (mt, nt), lambda: qk_piece(mt, nt))

                    def v_unit(b):
                        return ("v", b, lambda: v_block(b))

                    def proj_unit(mt):
                        return ("proj", mt, lambda: proj_mtile(mt))

                    def run_unit(u):
                        if u[0] == "qk":
                            qk_done.add(u[1])
                        u[2]()

                    def filler_step(n=2):
                        for _ in range(n):
                            if units:
                                run_unit(units.pop(0))

                    def ensure_qk(mt, nt):
                        while (mt, nt) not in qk_done and units:
                            run_unit(units.pop(0))

                    inflight = []   # av2 groups awaiting back2
                    pend = []       # scored pairs awaiting their AV matmuls
                    pair_n = [0]
                    # 3:1 gpsimd:vector for the exp(bias) multiply (SBUF-only op)
                    mul_cycle = [nc.gpsimd, nc.gpsimd, nc.vector, nc.gpsimd]

                    def flush_av():
                        # emit AV matmuls for the two OLDEST scored pairs —
                        # their expb-muls got a full slot of slack, so the
                        # FIFO PE queue never head-blocks on the softmax chain
                        (h0, b0, e0), (h1, b1, e1) = pend.pop(0), pend.pop(0)
                        av2 = ps_av_pool.tile([128, 2, 512], F32, name="av2", tag="av2")
                        attn_av(h0, b0, av2[:, 0, 0:2 * N], e0)
                        attn_av(h1, b1, av2[:, 1, 0:2 * N], e1)
                        inflight.append((h0, b0, av2))

                    def flush_back():
                        hp, b, av2 = inflight.pop(0)
                        attn_back2(hp, b, av2)
                        if hp == H // 2 - 1:
                            units.extend(proj_unit(mt) for mt in _PROJ_OF_GROUP[b // 2])

                    # PE warm-up while DMAs land: keeps HAM at 2.4 GHz
                    if WARMUP:
                        pw = ps_qk.tile([128, QK_TW], F32, name="psqk", tag="psqk")
                        for w in range(WARMUP):
                            nc.tensor.matmul(
                                pw, lhsT=junk_sb[:, 0:128], rhs=junk_sb[:, 0:QK_TW],
                                start=(w == 0), stop=(w == WARMUP - 1),
                            )

                    # ---- prologue: group 0's first head pair + V -----------
                    for mt in range(2):
                        qk_piece(mt, 0)
                        qk_done.add((mt, 0))
                    v_block(0)
                    v_block(1)
                    units.extend(qk_unit(mt, 0) for mt in range(2, 12))

                    # ---- main loop: batch-pair groups ----------------------
                    for g in range(QK_NT):
                        if g + 1 < QK_NT:
                            nxt = [qk_unit(mt, g + 1) for mt in range(4)]
                            nxt.append(v_unit(2 * (g + 1)))
                            nxt.append(v_unit(2 * (g + 1) + 1))
                            nxt += [qk_unit(mt, g + 1) for mt in range(4, 12)]
                            units.extend(nxt)
                        for hp in range(H // 2):
                            for fire in _gates.get((g, hp), ()):
                                fire()
                            ensure_qk(2 * hp, g)
                            ensure_qk(2 * hp + 1, g)
                            for db in range(2):
                                mul_eng = mul_cycle[pair_n[0] % len(mul_cycle)]
                                pair_n[0] += 1
                                pend.append((hp, 2 * g + db,
                                             attn_scores(hp, 2 * g + db, mul_eng)))
                            if len(inflight) >= 2:
                                flush_back()
                            filler_step(2)
                            if len(pend) >= 4:
                                flush_av()

                    # ---- drain --------------------------------------------
                    while len(pend) >= 2:
                        flush_av()
                        if len(inflight) >= 2:
                            flush_back()
                            filler_step(1)
                    while inflight:
                        flush_back()
                        filler_step(2)
                    while units:
                        run_unit(units.pop(0))

    nc.compile()
    return nc


def _bf16(a):
    return np.ascontiguousarray(a).astype(ml_dtypes.bfloat16)


def _host_prep(qkv_w, proj_w, rel_pos_table, rel_pos_index):
    """Shared (core-independent) SBUF images."""
    # exp of the transposed per-head bias, laid out as the kernel's score
    # tiles, two heads (one pair) side by side: head hh of pair hp occupies
    # columns hh*2N + [0, 2N): j-chunk0 rows 0:128, j-chunk1 rows 0:69 with
    # rows 69:128 zeroed (kills padded key rows).
    bias = rel_pos_table[rel_pos_index.reshape(-1)].reshape(N, N, H)  # [i, j, h]
    expb = np.zeros((128, H // 2, 4 * N), dtype=np.float32)
    eb = np.exp(bias)
    for h in range(H):
        ebT = eb[:, :, h].T  # [j, i]
        base = (h % 2) * 2 * N
        expb[:, h // 2, base:base + N] = ebT[0:128, :]
        expb[0:N - 128, h // 2, base + N:base + 2 * N] = ebT[128:N, :]
    expb16 = _bf16(expb)

    # wqk image [p, mt, kt, c], m-tile order [Q0,K0,Q1,K1,...]
    wqkT = np.ascontiguousarray(qkv_w[0:2 * C].T)            # [768, 1536]
    img = wqkT.reshape(KT, 128, 2, KT, 128).transpose(1, 3, 2, 0, 4)  # p,hp,s,kt,c
    wqk16 = _bf16(img.reshape(128, 12, KT, 128))

    wv16 = _bf16(qkv_w[2 * C:3 * C].T.reshape(KT, 128, C).transpose(1, 0, 2))
    wp16 = _bf16(proj_w.T.reshape(KT, 128, C).transpose(1, 0, 2))
    return wqk16, wv16, wp16, expb16


def _core_in_map(x, shared, core, has_q, has_v, has_p,
                 q_bias=None, v_bias=None, proj_b=None):
    wqk16, wv16, wp16, expb16 = shared
    xs = x[core * B_LOC:(core + 1) * B_LOC]                  # [8, 197, 768]
    # chunk-major image [p, chunk, kt, t'] so each chunk DMA is contiguous
    xT = _bf16(xs.reshape(NTOK, C).T.reshape(KT, 128, QK_NT, QK_TW)
               .transpose(1, 2, 0, 3))
    m = {"xT": xT, "wqk": wqk16, "wv": wv16, "wp": wp16, "expb": expb16}
    if has_q:
        m["qb"] = np.ascontiguousarray(q_bias.reshape(KT, 128).T)
    if has_v:
        m["vb"] = _bf16(v_bias.reshape(1, C))
    if has_p:
        m["pb"] = _bf16(proj_b.reshape(1, C))
    return m


def kernel(x, qkv_w, q_bias, v_bias, rel_pos_table, proj_w, proj_b, rel_pos_index):
    x = np.asarray(x, dtype=np.float32)
    qkv_w = np.asarray(qkv_w, dtype=np.float32)
    q_bias = np.asarray(q_bias, dtype=np.float32)
    v_bias = np.asarray(v_bias, dtype=np.float32)
    rel_pos_table = np.asarray(rel_pos_table, dtype=np.float32)
    proj_w = np.asarray(proj_w, dtype=np.float32)
    proj_b = np.asarray(proj_b, dtype=np.float32)
    rel_pos_index = np.asarray(rel_pos_index)

    has_q = bool(np.any(q_bias != 0))
    has_v = bool(np.any(v_bias != 0))
    has_p = bool(np.any(proj_b != 0))

    key = (has_v, has_p, has_q)
    if key not in _CACHE:
        _CACHE[key] = _build(*key)
    nc = _CACHE[key]

    shared = _host_prep(qkv_w, proj_w, rel_pos_table, rel_pos_index)
    in_maps = [
        _core_in_map(x, shared, c, has_q, has_v, has_p, q_bias, v_bias, proj_b)
        for c in range(N_CORES)
    ]

    res = bass_utils.run_bass_kernel_spmd(nc, in_maps, core_ids=list(range(N_CORES)))
    out = np.empty((B, N, C), dtype=np.float32)
    for c in range(N_CORES):
        out[c * B_LOC:(c + 1) * B_LOC] = np.asarray(
            res.results[c]["out"], dtype=np.float32).reshape(B_LOC, N, C)
    return out


# revision 15
# speedup vs baseline: 1.1018x; 1.0169x over previous
"""Trainium2 Bass kernel for ViT-style attention with relative position bias.

Module (per batch b, head h):
    qkv = x @ qkv_w.T + cat(q_bias, 0, v_bias)
    attn = softmax(scale * q @ k.T + bias[h])          bias = rel_pos_table[rel_pos_index]
    out  = (attn @ v) @ proj_w.T + proj_b

Distribution: pure data-parallel over batch — 8 NeuronCores x 8 batches each,
no collectives. Each core runs an identical SPMD program on its batch shard.

Device-side layout strategy (per core, all intermediates SBUF-resident bf16):
  - Host packs every input as its exact SBUF image so each tensor loads with
    a few large fully-contiguous DMAs, spread over four queues (sync: xT,
    scalar: wqk, vector: wv+wp, gpsimd: expb) with the first-needed chunks
    split in half so compute can start earlier.
  - While the first DMAs are in flight the PE runs warm-up matmuls on a junk
    tile so the HAM clock gate is at 2.4 GHz when real work starts.
  - qk^T matmul produces Q^T/K^T feature-major [64, tokens] slices directly,
    m-tile order interleaved [Q0,K0,Q1,K1,...].
  - V is token-major [tokens, 64] (the AV contraction needs keys on the
    partition axis), with a ones column block per head so the AV matmul also
    emits the softmax denominator rows for free.
  - Scores are computed transposed, S^T[j, i] = K[j] . Q[i]; softmax uses
    exp(s) * exp(bias) (no max-subtraction: logits are O(3), fp32/bf16 safe);
    exp(bias) is an input-derived constant computed host-side.
  - AV^T [64+64, 197] = [V_h | ones]^T @ expS^T is feature-major, feeding the
    proj matmul without transposes; the replicated denominator lands in
    partitions 64:128 and is reciprocal'd straight out of PSUM.
  - Schedule is batch-pair-group-outer: for each 394-token group (2 batches)
    all 12 heads' scores/AV run while the NEXT group's qk^T/V matmuls fill PE
    slack between attention pairs, and the PREVIOUS group's proj m-tiles
    stream out as soon as its last head is normalized. This spreads proj
    uniformly instead of bunching it in a drain phase at the end.
  - proj runs nt-outer so its two PSUM tiles retire alternately and the
    2-deep psum ring never head-blocks the PE.
  - Elementwise work is pinned per-engine (EXP: scalar, expb-mul mostly
    gpsimd, recip+normalize: vector, evacs balanced).
"""

import os
import numpy as np
import ml_dtypes

import concourse.bass as bass
import concourse.bacc as bacc
import concourse.mybir as mybir
import concourse.tile as tile
from concourse import bass_utils

F32 = mybir.dt.float32
BF16 = mybir.dt.bfloat16

N_CORES = 8
B = 64
B_LOC = B // N_CORES          # 8 batches per core
N = 197                       # tokens per batch
C = 768
H = 12
HD = 64
SCALE = HD ** -0.5
NTOK = B_LOC * N              # 1576
NPAD = 1664                   # 13 * 128 (qkT/aoT column allocation)
KT = 6                        # 768 / 128 contraction tiles
QK_NT = 4                     # n-tiles over tokens (= batch-pair groups)
QK_TW = NTOK // QK_NT         # 394

_CACHE = {}
WARMUP = int(os.environ.get("K_WARMUP", "20"))
OUTF32 = os.environ.get("K_OUTF32", "0") == "1"

# proj m-tile mt is ready once group (t_end-1)//QK_TW has all heads done
_PROJ_OF_GROUP = [[] for _ in range(QK_NT)]
for _mt in range(NPAD // 128):
    _t_end = min((_mt + 1) * 128, NTOK)
    _PROJ_OF_GROUP[(_t_end - 1) // QK_TW].append(_mt)


def _build(has_v_bias, has_p_bias, has_q_bias):
    nc = bacc.Bacc("TRN2", target_bir_lowering=False, debug=False)

    xT_d = nc.dram_tensor("xT", [128, QK_NT, KT, QK_TW], BF16, kind="ExternalInput")
    wqk_d = nc.dram_tensor("wqk", [128, 12, KT, 128], BF16, kind="ExternalInput")
    wv_d = nc.dram_tensor("wv", [128, KT, C], BF16, kind="ExternalInput")
    wp_d = nc.dram_tensor("wp", [128, KT, C], BF16, kind="ExternalInput")
    expb_d = nc.dram_tensor("expb", [128, H // 2, 4 * N], BF16, kind="ExternalInput")
    if has_q_bias:
        qb_d = nc.dram_tensor("qb", [128, KT], F32, kind="ExternalInput")
    if has_v_bias:
        vb_d = nc.dram_tensor("vb", [1, C], BF16, kind="ExternalInput")
    if has_p_bias:
        pb_d = nc.dram_tensor("pb", [1, C], BF16, kind="ExternalInput")
    out_d = nc.dram_tensor("out", [NTOK, C], F32 if OUTF32 else BF16, kind="ExternalOutput")

    with tile.TileContext(nc) as tc:
        with (
            tc.tile_pool(name="singles", bufs=1) as singles,
            tc.tile_pool(name="expwork", bufs=4) as expwork,
            tc.tile_pool(name="normwork", bufs=3) as normwork,
            tc.tile_pool(name="outstage", bufs=3) as outstage,
        ):
            # ---- persistent SBUF tensors ----
            wp_sb = singles.tile([128, KT, C], BF16)
            expb_sb = singles.tile([128, H // 2, 4 * N], BF16)
            # m-tile order [Q0, K0, Q1, K1, ...] to match wqk DMA chunks
            qkT_sb = singles.tile([128, 12, NPAD], BF16)
            # per-head blocks [V_h 64 | ones 64]; the ones half makes each AV
            # matmul replicate the softmax denominator into PSUM partitions
            # 64:128 (lhsT = [V_h | ones], 128 contiguous columns)
            V_sb = singles.tile([128, 2 * B_LOC, H, 2 * HD], BF16)
            aoT_sb = singles.tile([128, KT, NPAD], BF16)      # attn-out^T, proj stationary
            junk_sb = singles.tile([128, 512], BF16)

            if has_v_bias or has_p_bias:
                ones_row = singles.tile([1, NPAD], BF16)
            if has_p_bias:
                pb_sb = singles.tile([1, C], BF16)

            with tc.tile_pool(name="wqkpool", bufs=1) as wqkpool:
                wqk_sb = wqkpool.tile([128, 12, KT, 128], BF16)
                xT_sb = wqkpool.tile([128, QK_NT, KT, QK_TW], BF16)

                with (
                    tc.tile_pool(name="ps_qk", bufs=2, space="PSUM") as ps_qk,
                    tc.tile_pool(name="ps_s", bufs=1, space="PSUM") as ps_s_pool,
                    tc.tile_pool(name="ps_av2", bufs=2, space="PSUM") as ps_av_pool,
                    tc.tile_pool(name="xvpool", bufs=1) as xvpool,
                ):
                    wv_sb = xvpool.tile([128, KT, C], BF16)
                    if has_v_bias:
                        vb_sb2 = xvpool.tile([1, C], BF16)
                    else:
                        vb_sb2 = None

                    # ---- engine-ordered preamble ----------------------------
                    # ~8 DMA transfers can be in flight (HWDGE+SWDGE lanes) and
                    # concurrent transfers fair-share HBM bandwidth, so bulk
                    # loads would starve the critical first-group tensors.
                    # Wave 1 (enqueued here) carries only what the first ~15us
                    # of compute needs; the rest is enqueued from the scalar
                    # engine's FIFO at later schedule points (see _gates), so
                    # its EXP pacing acts as a wall-clock gate.
                    nc.vector.memset(junk_sb, 1.0)           # warm-up dep
                    # sync queue: xT chunk 0 in two halves (smaller transfers
                    # finish earlier under fair-share)
                    nc.sync.dma_start(out=xT_sb[:, 0, 0:3], in_=xT_d.ap()[:, 0, 0:3])
                    nc.sync.dma_start(out=xT_sb[:, 0, 3:6], in_=xT_d.ap()[:, 0, 3:6])
                    # scalar queue: wqk pair 0 in two halves
                    nc.scalar.dma_start(out=wqk_sb[:, 0:2, 0:3], in_=wqk_d.ap()[:, 0:2, 0:3])
                    nc.scalar.dma_start(out=wqk_sb[:, 0:2, 3:6], in_=wqk_d.ap()[:, 0:2, 3:6])
                    # gpsimd queue: wv kt-halves (contiguous), first expb pairs,
                    # then xT chunk 1 (needed by the next group's qk/V filler)
                    nc.gpsimd.dma_start(out=wv_sb[:, 0:3], in_=wv_d.ap()[:, 0:3])
                    nc.gpsimd.dma_start(out=wv_sb[:, 3:6], in_=wv_d.ap()[:, 3:6])
                    nc.gpsimd.dma_start(out=expb_sb[:, 0:1], in_=expb_d.ap()[:, 0:1])
                    nc.gpsimd.dma_start(out=expb_sb[:, 1:2], in_=expb_d.ap()[:, 1:2])
                    if has_v_bias:
                        nc.gpsimd.dma_start(out=vb_sb2, in_=vb_d.ap())
                    if has_q_bias:
                        qb_sb = singles.tile([128, KT], F32)
                        nc.sync.dma_start(out=qb_sb, in_=qb_d.ap())
                    if has_p_bias:
                        nc.gpsimd.dma_start(out=pb_sb, in_=pb_d.ap())

                    # memsets on vector (no early work until the first qk evac
                    # ~12us in); the late V-ones halves go via gates so they
                    # don't delay the first evacs
                    if has_v_bias or has_p_bias:
                        nc.vector.memset(ones_row, 1.0)
                    nc.vector.memset(V_sb[:, 0:4, :, HD:2 * HD], 1.0)
                    # score lhsT for the last batch reads K-mtile cols past NTOK,
                    # and odd batches read a 59-col overhang into the next
                    # group's columns (garbage rows, masked by expb zero rows)
                    # — both must be initialized before first read.
                    for m in range(12):
                        if m % 2 == 1:
                            nc.vector.memset(qkT_sb[:, m, NTOK:NPAD], 0.0)
                            for g in range(1, QK_NT):
                                nc.vector.memset(
                                    qkT_sb[:, m, g * QK_TW:g * QK_TW + 2 * 128 - N], 0.0)
                    # proj reads padded token columns of attn-out^T
                    nc.vector.memset(aoT_sb[:, :, NTOK:NPAD], 0.0)
                    nc.vector.memset(V_sb[:, 4:8, :, HD:2 * HD], 1.0)

                    # staged bulk DMA enqueues, fired from the gpsimd FIFO at
                    # (group, head-pair) points — gpsimd blocks on the first
                    # softmax multiply, so these genuinely wait until the
                    # schedule reaches that point before competing for HBM
                    def _wqk_pair(hp):
                        return lambda: nc.gpsimd.dma_start(
                            out=wqk_sb[:, 2 * hp:2 * hp + 2],
                            in_=wqk_d.ap()[:, 2 * hp:2 * hp + 2])

                    def _expb_pair(hp):
                        return lambda: nc.gpsimd.dma_start(
                            out=expb_sb[:, hp:hp + 1], in_=expb_d.ap()[:, hp:hp + 1])

                    def _xT_half(c, h):
                        return lambda: nc.gpsimd.dma_start(
                            out=xT_sb[:, c, 3 * h:3 * h + 3],
                            in_=xT_d.ap()[:, c, 3 * h:3 * h + 3])

                    def _wp_half(h):
                        return lambda: nc.gpsimd.dma_start(
                            out=wp_sb[:, 3 * h:3 * h + 3], in_=wp_d.ap()[:, 3 * h:3 * h + 3])

                    def _vones(lo, hi):
                        return lambda: nc.vector.memset(
                            V_sb[:, lo:hi, :, HD:2 * HD], 1.0)

                    _gates = {
                        (0, 0): [_wqk_pair(1), _wqk_pair(2), _xT_half(1, 0)],
                        (0, 1): [_wqk_pair(3), _xT_half(1, 1), _vones(8, 12)],
                        (0, 2): [_wqk_pair(4), _expb_pair(2)],
                        (0, 3): [_wqk_pair(5), _expb_pair(3), _vones(12, 16)],
                        (0, 4): [_expb_pair(4), _expb_pair(5)],
                        (0, 5): [_xT_half(2, 0), _wp_half(0)],
                        (1, 0): [_xT_half(2, 1), _wp_half(1)],
                        (1, 1): [_xT_half(3, 0)],
                        (1, 2): [_xT_half(3, 1)],
                    }

                    evac_n = [0]
                    # gpsimd cannot access PSUM — evacs go scalar/vector only
                    evac_engines = [nc.scalar, nc.vector]

                    def evac_copy(out, in_):
                        e = evac_engines[evac_n[0] % len(evac_engines)]
                        evac_n[0] += 1
                        if e is nc.scalar:
                            e.copy(out=out, in_=in_)
                        else:
                            e.tensor_copy(out=out, in_=in_)

                    def qk_piece(mt, nt):
                        pq = ps_qk.tile([128, QK_TW], F32, name="psqk", tag="psqk")
                        for kt in range(KT):
                            nc.tensor.matmul(
                                pq,
                                lhsT=wqk_sb[:, mt, kt, :],
                                rhs=xT_sb[:, nt, kt, :],
                                start=(kt == 0), stop=(kt == KT - 1),
                            )
                        dst = qkT_sb[:, mt, nt * QK_TW:(nt + 1) * QK_TW]
                        if has_q_bias and mt % 2 == 0:
                            nc.any.tensor_scalar_add(dst, pq, qb_sb[:, mt // 2:mt // 2 + 1])
                        else:
                            nc.vector.tensor_copy(out=dst, in_=pq)

                    def v_block(b):
                        # token-major V for one batch, psum borrowed from ps_qk
                        for jc in range(2):
                            m = 128 if jc == 0 else N - 128
                            tok0 = b * N + jc * 128
                            tc_, to_ = tok0 // QK_TW, tok0 % QK_TW
                            for nt in range(2):
                                pv = ps_qk.tile([128, QK_TW], F32, name="psqk", tag="psqk")
                                for kt in range(KT):
                                    nc.tensor.matmul(
                                        pv[0:m, 0:C // 2],
                                        lhsT=xT_sb[:, tc_, kt, to_:to_ + m],
                                        rhs=wv_sb[:, kt, nt * (C // 2):(nt + 1) * (C // 2)],
                                        start=(kt == 0),
                                        stop=(kt == KT - 1 and not has_v_bias),
                                    )
                                if has_v_bias:
                                    nc.tensor.matmul(
                                        pv[0:m, 0:C // 2],
                                        lhsT=ones_row[:, tok0:tok0 + m],
                                        rhs=vb_sb2[:, nt * (C // 2):(nt + 1) * (C // 2)],
                                        start=False, stop=True,
                                    )
                                evac_copy(
                                    V_sb[0:m, b * 2 + jc, nt * 6:(nt + 1) * 6, 0:HD],
                                    pv[0:m, 0:C // 2].rearrange("p (g f) -> p g f", g=KT),
                                )

                    def attn_scores(hp, b, mul_eng):
                        q0 = b * N
                        # Both heads' scores in one 2-bank PSUM tile (head hh at
                        # columns hh*512 + [0, 394)) so one bank-hopping AP
                        # covers the pair in a single EXP. Even/odd heads live
                        # at base partitions 0/64, so their k=64 score matmuls
                        # occupy disjoint PE row groups and overlap.
                        ps_s2 = ps_s_pool.tile([128, 1024], F32, name="ps_s2", tag="ps_s2")
                        for jc in range(2):
                            for hh in range(2):
                                h = hp * 2 + hh
                                pbase = (h % 2) * 64
                                nc.tensor.matmul(
                                    ps_s2[:, hh * 512 + jc * N:hh * 512 + (jc + 1) * N],
                                    lhsT=qkT_sb[pbase:pbase + 64, 2 * hp + 1,
                                                q0 + jc * 128:q0 + jc * 128 + 128],
                                    rhs=qkT_sb[pbase:pbase + 64, 2 * hp, q0:q0 + N],
                                    start=(jc == 0), stop=(jc == 1),
                                )
                        expS2 = expwork.tile([128, 4 * N], BF16, tag="expS2")
                        nc.scalar.activation(
                            expS2.rearrange("p (g w) -> p g w", w=2 * N),
                            ps_s2.rearrange("p (g w) -> p g w", w=512)[:, :, 0:2 * N],
                            mybir.ActivationFunctionType.Exp, scale=SCALE,
                        )
                        expST2 = expwork.tile([128, 4 * N], BF16, tag="expST2")
                        mul_eng.tensor_mul(expST2, expS2, expb_sb[:, hp, :])
                        return expST2

                    def attn_av(hp, b, av, expST2):
                        for hh in range(2):
                            h = hp * 2 + hh
                            for jc in range(2):
                                jm = 128 if jc == 0 else N - 128
                                nc.tensor.matmul(
                                    av[:, hh * N:(hh + 1) * N],
                                    lhsT=V_sb[0:jm, b * 2 + jc, h, :],
                                    rhs=expST2[0:jm, hh * 2 * N + jc * N:hh * 2 * N + (jc + 1) * N],
                                    start=(hh == 0 and jc == 0),
                                    stop=(hh == 1 and jc == 1),
                                )

                    def attn_back2(hp, b, av2):
                        # one copy/recip/mul set covers pairs (hp,b) and (hp,b+1)
                        denom2 = normwork.tile([64, 2, 2 * N], F32, tag="denom")
                        nc.scalar.copy(out=denom2, in_=av2[64:128, :, 0:2 * N])
                        recipB2 = normwork.tile([64, 2, 2 * N], F32, tag="recipB")
                        nc.vector.reciprocal_approx_fast(recipB2, denom2)
                        for hh in range(2):
                            h = hp * 2 + hh
                            dst = aoT_sb[(h % 2) * 64:(h % 2) * 64 + 64, h // 2,
                                         b * N:(b + 2) * N]
                            nc.vector.tensor_mul(
                                dst.rearrange("p (n w) -> p n w", n=2),
                                av2[0:HD, :, hh * N:(hh + 1) * N],
                                recipB2[:, :, hh * N:(hh + 1) * N],
                            )

                    out_dma = [nc.sync, nc.scalar]

                    def proj_mtile(mt):
                        rows = min(128, NTOK - mt * 128)
                        stage = outstage.tile([128, C], F32 if OUTF32 else BF16, tag="stage")
                        # nt-outer: the two psum tiles retire alternately so the
                        # 2-deep ring never stalls the PE between m-tiles
                        for nt in range(2):
                            ps = ps_qk.tile([128, QK_TW], F32, name="psqk", tag="psqk")
                            for kt in range(KT):
                                nc.tensor.matmul(
                                    ps[:, 0:C // 2],
                                    lhsT=aoT_sb[:, kt, mt * 128:(mt + 1) * 128],
                                    rhs=wp_sb[:, kt, nt * (C // 2):(nt + 1) * (C // 2)],
                                    start=(kt == 0),
                                    stop=(kt == KT - 1 and not has_p_bias),
                                )
                            if has_p_bias:
                                nc.tensor.matmul(
                                    ps[:, 0:C // 2],
                                    lhsT=ones_row[:, mt * 128:(mt + 1) * 128],
                                    rhs=pb_sb[:, nt * (C // 2):(nt + 1) * (C // 2)],
                                    start=False, stop=True,
                                )
                            if nt == 0:
                                nc.scalar.copy(out=stage[:, 0:C // 2], in_=ps[:, 0:C // 2])
                            else:
                                nc.vector.tensor_copy(out=stage[:, C // 2:C], in_=ps[:, 0:C // 2])
                        out_dma[0 if OUTF32 else mt % 2].dma_start(
                            out=out_d.ap()[mt * 128:mt * 128 + rows, :], in_=stage[0:rows, :]
                        )

                    # ---- filler unit scheduler ------------------------------
                    # Units are dense work (qk pieces, V blocks, proj m-tiles,
                    # deferred memsets) emitted between attention pairs so the
                    # FIFO PE queue never head-blocks on the softmax chain.
                    units = []
                    qk_done = set()

                    def qk_unit(mt, nt):
                        return ("qk", (mt, nt), lambda: qk_piece(mt, nt))

                    def v_unit(b):
                        return ("v", b, lambda: v_block(b))

                    def proj_unit(mt):
                        return ("proj", mt, lambda: proj_mtile(mt))

                    def run_unit(u):
                        if u[0] == "qk":
                            qk_done.add(u[1])
                        u[2]()

                    def filler_step(n=2):
                        for _ in range(n):
                            if units:
                                run_unit(units.pop(0))

                    def ensure_qk(mt, nt):
                        while (mt, nt) not in qk_done and units:
                            run_unit(units.pop(0))

                    inflight = []   # av2 groups awaiting back2
                    pend = []       # scored pairs awaiting their AV matmuls
                    pair_n = [0]
                    # 3:1 gpsimd:vector for the exp(bias) multiply (SBUF-only op)
                    mul_cycle = [nc.gpsimd, nc.gpsimd, nc.vector, nc.gpsimd]

                    def flush_av():
                        # emit AV matmuls for the two OLDEST scored pairs —
                        # their expb-muls got a full slot of slack, so the
                        # FIFO PE queue never head-blocks on the softmax chain
                        (h0, b0, e0), (h1, b1, e1) = pend.pop(0), pend.pop(0)
                        av2 = ps_av_pool.tile([128, 2, 512], F32, name="av2", tag="av2")
                        attn_av(h0, b0, av2[:, 0, 0:2 * N], e0)
                        attn_av(h1, b1, av2[:, 1, 0:2 * N], e1)
                        inflight.append((h0, b0, av2))

                    def flush_back():
                        hp, b, av2 = inflight.pop(0)
                        attn_back2(hp, b, av2)
                        if hp == H // 2 - 1:
                            units.extend(proj_unit(mt) for mt in _PROJ_OF_GROUP[b // 2])

                    # PE warm-up while DMAs land: keeps HAM at 2.4 GHz
                    if WARMUP:
                        pw = ps_qk.tile([128, QK_TW], F32, name="psqk", tag="psqk")
                        for w in range(WARMUP):
                            nc.tensor.matmul(
                                pw, lhsT=junk_sb[:, 0:128], rhs=junk_sb[:, 0:QK_TW],
                                start=(w == 0), stop=(w == WARMUP - 1),
                            )

                    # ---- prologue: group 0's first head pair + V -----------
                    for mt in range(2):
                        qk_piece(mt, 0)
                        qk_done.add((mt, 0))
                    v_block(0)
                    v_block(1)
                    units.extend(qk_unit(mt, 0) for mt in range(2, 12))

                    # ---- main loop: batch-pair groups ----------------------
                    for g in range(QK_NT):
                        if g + 1 < QK_NT:
                            nxt = [qk_unit(mt, g + 1) for mt in range(4)]
                            nxt.append(v_unit(2 * (g + 1)))
                            nxt.append(v_unit(2 * (g + 1) + 1))
                            nxt += [qk_unit(mt, g + 1) for mt in range(4, 12)]
                            units.extend(nxt)
                        for hp in range(H // 2):
                            for fire in _gates.get((g, hp), ()):
                                fire()
                            ensure_qk(2 * hp, g)
                            ensure_qk(2 * hp + 1, g)
                            for db in range(2):
                                mul_eng = mul_cycle[pair_n[0] % len(mul_cycle)]
                                pair_n[0] += 1
                                pend.append((hp, 2 * g + db,
                                             attn_scores(hp, 2 * g + db, mul_eng)))
                            if len(inflight) >= 2:
                                flush_back()
                            filler_step(2)
                            if len(pend) >= 4:
                                flush_av()

                    # ---- drain --------------------------------------------
                    while len(pend) >= 2:
                        flush_av()
                        if len(inflight) >= 2:
                            flush_back()
                            filler_step(1)
                    while inflight:
                        flush_back()
                        filler_step(2)
                    while units:
                        run_unit(units.pop(0))

    nc.compile()
    return nc


def _bf16(a):
    return np.ascontiguousarray(a).astype(ml_dtypes.bfloat16)


def _host_prep(qkv_w, proj_w, rel_pos_table, rel_pos_index):
    """Shared (core-independent) SBUF images."""
    # exp of the transposed per-head bias, laid out as the kernel's score
    # tiles, two heads (one pair) side by side: head hh of pair hp occupies
    # columns hh*2N + [0, 2N): j-chunk0 rows 0:128, j-chunk1 rows 0:69 with
    # rows 69:128 zeroed (kills padded key rows).
    bias = rel_pos_table[rel_pos_index.reshape(-1)].reshape(N, N, H)  # [i, j, h]
    expb = np.zeros((128, H // 2, 4 * N), dtype=np.float32)
    eb = np.exp(bias)
    for h in range(H):
        ebT = eb[:, :, h].T  # [j, i]
        base = (h % 2) * 2 * N
        expb[:, h // 2, base:base + N] = ebT[0:128, :]
        expb[0:N - 128, h // 2, base + N:base + 2 * N] = ebT[128:N, :]
    expb16 = _bf16(expb)

    # wqk image [p, mt, kt, c], m-tile order [Q0,K0,Q1,K1,...]
    wqkT = np.ascontiguousarray(qkv_w[0:2 * C].T)            # [768, 1536]
    img = wqkT.reshape(KT, 128, 2, KT, 128).transpose(1, 3, 2, 0, 4)  # p,hp,s,kt,c
    wqk16 = _bf16(img.reshape(128, 12, KT, 128))

    wv16 = _bf16(qkv_w[2 * C:3 * C].T.reshape(KT, 128, C).transpose(1, 0, 2))
    wp16 = _bf16(proj_w.T.reshape(KT, 128, C).transpose(1, 0, 2))
    return wqk16, wv16, wp16, expb16


def _core_in_map(x, shared, core, has_q, has_v, has_p,
                 q_bias=None, v_bias=None, proj_b=None):
    wqk16, wv16, wp16, expb16 = shared
    xs = x[core * B_LOC:(core + 1) * B_LOC]                  # [8, 197, 768]
    # chunk-major image [p, chunk, kt, t'] so each chunk DMA is contiguous
    xT = _bf16(xs.reshape(NTOK, C).T.reshape(KT, 128, QK_NT, QK_TW)
               .transpose(1, 2, 0, 3))
    m = {"xT": xT, "wqk": wqk16, "wv": wv16, "wp": wp16, "expb": expb16}
    if has_q:
        m["qb"] = np.ascontiguousarray(q_bias.reshape(KT, 128).T)
    if has_v:
        m["vb"] = _bf16(v_bias.reshape(1, C))
    if has_p:
        m["pb"] = _bf16(proj_b.reshape(1, C))
    return m


def kernel(x, qkv_w, q_bias, v_bias, rel_pos_table, proj_w, proj_b, rel_pos_index):
    x = np.asarray(x, dtype=np.float32)
    qkv_w = np.asarray(qkv_w, dtype=np.float32)
    q_bias = np.asarray(q_bias, dtype=np.float32)
    v_bias = np.asarray(v_bias, dtype=np.float32)
    rel_pos_table = np.asarray(rel_pos_table, dtype=np.float32)
    proj_w = np.asarray(proj_w, dtype=np.float32)
    proj_b = np.asarray(proj_b, dtype=np.float32)
    rel_pos_index = np.asarray(rel_pos_index)

    has_q = bool(np.any(q_bias != 0))
    has_v = bool(np.any(v_bias != 0))
    has_p = bool(np.any(proj_b != 0))

    key = (has_v, has_p, has_q)
    if key not in _CACHE:
        _CACHE[key] = _build(*key)
    nc = _CACHE[key]

    shared = _host_prep(qkv_w, proj_w, rel_pos_table, rel_pos_index)
    in_maps = [
        _core_in_map(x, shared, c, has_q, has_v, has_p, q_bias, v_bias, proj_b)
        for c in range(N_CORES)
    ]

    res = bass_utils.run_bass_kernel_spmd(nc, in_maps, core_ids=list(range(N_CORES)))
    out = np.empty((B, N, C), dtype=np.float32)
    for c in range(N_CORES):
        out[c * B_LOC:(c + 1) * B_LOC] = np.asarray(
            res.results[c]["out"], dtype=np.float32).reshape(B_LOC, N, C)
    return out


# revision 16
# speedup vs baseline: 1.1040x; 1.0020x over previous
"""Trainium2 Bass kernel for ViT-style attention with relative position bias.

Module (per batch b, head h):
    qkv = x @ qkv_w.T + cat(q_bias, 0, v_bias)
    attn = softmax(scale * q @ k.T + bias[h])          bias = rel_pos_table[rel_pos_index]
    out  = (attn @ v) @ proj_w.T + proj_b

Distribution: pure data-parallel over batch — 8 NeuronCores x 8 batches each,
no collectives. Each core runs an identical SPMD program on its batch shard.

Device-side layout strategy (per core, all intermediates SBUF-resident bf16):
  - Host packs every input as its exact SBUF image so each tensor loads with
    a few large fully-contiguous DMAs, spread over four queues (sync: xT,
    scalar: wqk, vector: wv+wp, gpsimd: expb) with the first-needed chunks
    split in half so compute can start earlier.
  - While the first DMAs are in flight the PE runs warm-up matmuls on a junk
    tile so the HAM clock gate is at 2.4 GHz when real work starts.
  - qk^T matmul produces Q^T/K^T feature-major [64, tokens] slices directly,
    m-tile order interleaved [Q0,K0,Q1,K1,...].
  - V is token-major [tokens, 64] (the AV contraction needs keys on the
    partition axis), with a ones column block per head so the AV matmul also
    emits the softmax denominator rows for free.
  - Scores are computed transposed, S^T[j, i] = K[j] . Q[i]; softmax uses
    exp(s) * exp(bias) (no max-subtraction: logits are O(3), fp32/bf16 safe);
    exp(bias) is an input-derived constant computed host-side.
  - AV^T [64+64, 197] = [V_h | ones]^T @ expS^T is feature-major, feeding the
    proj matmul without transposes; the replicated denominator lands in
    partitions 64:128 and is reciprocal'd straight out of PSUM.
  - Schedule is batch-pair-group-outer: for each 394-token group (2 batches)
    all 12 heads' scores/AV run while the NEXT group's qk^T/V matmuls fill PE
    slack between attention pairs, and the PREVIOUS group's proj m-tiles
    stream out as soon as its last head is normalized. This spreads proj
    uniformly instead of bunching it in a drain phase at the end.
  - proj runs nt-outer so its two PSUM tiles retire alternately and the
    2-deep psum ring never head-blocks the PE.
  - Elementwise work is pinned per-engine (EXP: scalar, expb-mul mostly
    gpsimd, recip+normalize: vector, evacs balanced).
"""

import os
import numpy as np
import ml_dtypes

import concourse.bass as bass
import concourse.bacc as bacc
import concourse.mybir as mybir
import concourse.tile as tile
from concourse import bass_utils

F32 = mybir.dt.float32
BF16 = mybir.dt.bfloat16

N_CORES = 8
B = 64
B_LOC = B // N_CORES          # 8 batches per core
N = 197                       # tokens per batch
C = 768
H = 12
HD = 64
SCALE = HD ** -0.5
NTOK = B_LOC * N              # 1576
NPAD = 1664                   # 13 * 128 (qkT/aoT column allocation)
KT = 6                        # 768 / 128 contraction tiles
QK_NT = 4                     # n-tiles over tokens (= batch-pair groups)
QK_TW = NTOK // QK_NT         # 394

_CACHE = {}
WARMUP = int(os.environ.get("K_WARMUP", "20"))
OUTF32 = os.environ.get("K_OUTF32", "0") == "1"

# proj m-tile mt is ready once group (t_end-1)//QK_TW has all heads done
_PROJ_OF_GROUP = [[] for _ in range(QK_NT)]
for _mt in range(NPAD // 128):
    _t_end = min((_mt + 1) * 128, NTOK)
    _PROJ_OF_GROUP[(_t_end - 1) // QK_TW].append(_mt)


def _build(has_v_bias, has_p_bias, has_q_bias):
    nc = bacc.Bacc("TRN2", target_bir_lowering=False, debug=False)

    xT_d = nc.dram_tensor("xT", [128, QK_NT, KT, QK_TW], BF16, kind="ExternalInput")
    wqk_d = nc.dram_tensor("wqk", [128, 12, KT, 128], BF16, kind="ExternalInput")
    wv_d = nc.dram_tensor("wv", [128, KT, C], BF16, kind="ExternalInput")
    wp_d = nc.dram_tensor("wp", [128, KT, C], BF16, kind="ExternalInput")
    expb_d = nc.dram_tensor("expb", [128, H // 2, 4 * N], BF16, kind="ExternalInput")
    if has_q_bias:
        qb_d = nc.dram_tensor("qb", [128, KT], F32, kind="ExternalInput")
    if has_v_bias:
        vb_d = nc.dram_tensor("vb", [1, C], BF16, kind="ExternalInput")
    if has_p_bias:
        pb_d = nc.dram_tensor("pb", [1, C], BF16, kind="ExternalInput")
    out_d = nc.dram_tensor("out", [NTOK, C], F32 if OUTF32 else BF16, kind="ExternalOutput")

    with tile.TileContext(nc) as tc:
        with (
            tc.tile_pool(name="singles", bufs=1) as singles,
            tc.tile_pool(name="expwork", bufs=4) as expwork,
            tc.tile_pool(name="normwork", bufs=3) as normwork,
            tc.tile_pool(name="outstage", bufs=3) as outstage,
        ):
            # ---- persistent SBUF tensors ----
            wp_sb = singles.tile([128, KT, C], BF16)
            expb_sb = singles.tile([128, H // 2, 4 * N], BF16)
            # m-tile order [Q0, K0, Q1, K1, ...] to match wqk DMA chunks
            qkT_sb = singles.tile([128, 12, NPAD], BF16)
            # per-head blocks [V_h 64 | ones 64]; the ones half makes each AV
            # matmul replicate the softmax denominator into PSUM partitions
            # 64:128 (lhsT = [V_h | ones], 128 contiguous columns)
            V_sb = singles.tile([128, 2 * B_LOC, H, 2 * HD], BF16)
            aoT_sb = singles.tile([128, KT, NPAD], BF16)      # attn-out^T, proj stationary
            junk_sb = singles.tile([128, 512], BF16)

            if has_v_bias or has_p_bias:
                ones_row = singles.tile([1, NPAD], BF16)
            if has_p_bias:
                pb_sb = singles.tile([1, C], BF16)

            with tc.tile_pool(name="wqkpool", bufs=1) as wqkpool:
                wqk_sb = wqkpool.tile([128, 12, KT, 128], BF16)
                xT_sb = wqkpool.tile([128, QK_NT, KT, QK_TW], BF16)

                with (
                    tc.tile_pool(name="ps_qk", bufs=2, space="PSUM") as ps_qk,
                    tc.tile_pool(name="ps_s", bufs=1, space="PSUM") as ps_s_pool,
                    tc.tile_pool(name="ps_av2", bufs=2, space="PSUM") as ps_av_pool,
                    tc.tile_pool(name="xvpool", bufs=1) as xvpool,
                ):
                    wv_sb = xvpool.tile([128, KT, C], BF16)
                    if has_v_bias:
                        vb_sb2 = xvpool.tile([1, C], BF16)
                    else:
                        vb_sb2 = None

                    # ---- engine-ordered preamble ----------------------------
                    # ~8 DMA transfers can be in flight (HWDGE+SWDGE lanes) and
                    # concurrent transfers fair-share HBM bandwidth, so bulk
                    # loads would starve the critical first-group tensors.
                    # Wave 1 (enqueued here) carries only what the first ~15us
                    # of compute needs; the rest is enqueued from the scalar
                    # engine's FIFO at later schedule points (see _gates), so
                    # its EXP pacing acts as a wall-clock gate.
                    nc.vector.memset(junk_sb, 1.0)           # warm-up dep
                    # sync queue: xT chunk 0 in two halves (smaller transfers
                    # finish earlier under fair-share)
                    nc.sync.dma_start(out=xT_sb[:, 0, 0:3], in_=xT_d.ap()[:, 0, 0:3])
                    nc.sync.dma_start(out=xT_sb[:, 0, 3:6], in_=xT_d.ap()[:, 0, 3:6])
                    # scalar queue: wqk pair 0 in two halves
                    nc.scalar.dma_start(out=wqk_sb[:, 0:2, 0:3], in_=wqk_d.ap()[:, 0:2, 0:3])
                    nc.scalar.dma_start(out=wqk_sb[:, 0:2, 3:6], in_=wqk_d.ap()[:, 0:2, 3:6])
                    # gpsimd queue: wv kt-halves (contiguous), first expb pairs,
                    # then xT chunk 1 (needed by the next group's qk/V filler)
                    nc.gpsimd.dma_start(out=wv_sb[:, 0:3], in_=wv_d.ap()[:, 0:3])
                    nc.gpsimd.dma_start(out=wv_sb[:, 3:6], in_=wv_d.ap()[:, 3:6])
                    nc.gpsimd.dma_start(out=expb_sb[:, 0:1], in_=expb_d.ap()[:, 0:1])
                    nc.gpsimd.dma_start(out=expb_sb[:, 1:2], in_=expb_d.ap()[:, 1:2])
                    if has_v_bias:
                        nc.gpsimd.dma_start(out=vb_sb2, in_=vb_d.ap())
                    if has_q_bias:
                        qb_sb = singles.tile([128, KT], F32)
                        nc.sync.dma_start(out=qb_sb, in_=qb_d.ap())
                    if has_p_bias:
                        nc.gpsimd.dma_start(out=pb_sb, in_=pb_d.ap())

                    # memsets on vector (no early work until the first qk evac
                    # ~12us in); the late V-ones halves go via gates so they
                    # don't delay the first evacs
                    if has_v_bias or has_p_bias:
                        nc.vector.memset(ones_row, 1.0)
                    nc.vector.memset(V_sb[:, 0:4, :, HD:2 * HD], 1.0)
                    # score lhsT for the last batch reads K-mtile cols past NTOK,
                    # and odd batches read a 59-col overhang into the next
                    # group's columns (garbage rows, masked by expb zero rows)
                    # — both must be initialized before first read.
                    for m in range(12):
                        if m % 2 == 1:
                            nc.vector.memset(qkT_sb[:, m, NTOK:NPAD], 0.0)
                            for g in range(1, QK_NT):
                                nc.vector.memset(
                                    qkT_sb[:, m, g * QK_TW:g * QK_TW + 2 * 128 - N], 0.0)
                    # proj reads padded token columns of attn-out^T
                    nc.vector.memset(aoT_sb[:, :, NTOK:NPAD], 0.0)
                    nc.vector.memset(V_sb[:, 4:8, :, HD:2 * HD], 1.0)

                    # staged bulk DMA enqueues, fired from the gpsimd FIFO at
                    # (group, head-pair) points — gpsimd blocks on the first
                    # softmax multiply, so these genuinely wait until the
                    # schedule reaches that point before competing for HBM
                    def _wqk_pair(hp):
                        return lambda: nc.gpsimd.dma_start(
                            out=wqk_sb[:, 2 * hp:2 * hp + 2],
                            in_=wqk_d.ap()[:, 2 * hp:2 * hp + 2])

                    def _expb_pair(hp):
                        return lambda: nc.gpsimd.dma_start(
                            out=expb_sb[:, hp:hp + 1], in_=expb_d.ap()[:, hp:hp + 1])

                    def _xT_half(c, h):
                        return lambda: nc.gpsimd.dma_start(
                            out=xT_sb[:, c, 3 * h:3 * h + 3],
                            in_=xT_d.ap()[:, c, 3 * h:3 * h + 3])

                    def _wp_half(h):
                        return lambda: nc.gpsimd.dma_start(
                            out=wp_sb[:, 3 * h:3 * h + 3], in_=wp_d.ap()[:, 3 * h:3 * h + 3])

                    def _vones(lo, hi):
                        return lambda: nc.vector.memset(
                            V_sb[:, lo:hi, :, HD:2 * HD], 1.0)

                    _gates = {
                        # (0,0) fires immediately (gpsimd FIFO only blocks at
                        # the first multiply) — keep it to the one pair the
                        # slot-0 filler needs so wave 1 stays small
                        (0, 0): [_wqk_pair(1)],
                        (0, 1): [_wqk_pair(2), _expb_pair(2), _vones(8, 12)],
                        (0, 2): [_wqk_pair(3), _expb_pair(3), _xT_half(1, 0)],
                        (0, 3): [_wqk_pair(4), _expb_pair(4), _xT_half(1, 1),
                                 _vones(12, 16)],
                        (0, 4): [_wqk_pair(5), _expb_pair(5)],
                        (0, 5): [_xT_half(2, 0), _wp_half(0)],
                        (1, 0): [_xT_half(2, 1), _wp_half(1)],
                        (1, 1): [_xT_half(3, 0)],
                        (1, 2): [_xT_half(3, 1)],
                    }

                    evac_n = [0]
                    # gpsimd cannot access PSUM — evacs go scalar/vector only
                    evac_engines = [nc.scalar, nc.vector]

                    def evac_copy(out, in_):
                        e = evac_engines[evac_n[0] % len(evac_engines)]
                        evac_n[0] += 1
                        if e is nc.scalar:
                            e.copy(out=out, in_=in_)
                        else:
                            e.tensor_copy(out=out, in_=in_)

                    def qk_piece(mt, nt):
                        pq = ps_qk.tile([128, QK_TW], F32, name="psqk", tag="psqk")
                        for kt in range(KT):
                            nc.tensor.matmul(
                                pq,
                                lhsT=wqk_sb[:, mt, kt, :],
                                rhs=xT_sb[:, nt, kt, :],
                                start=(kt == 0), stop=(kt == KT - 1),
                            )
                        dst = qkT_sb[:, mt, nt * QK_TW:(nt + 1) * QK_TW]
                        if has_q_bias and mt % 2 == 0:
                            nc.any.tensor_scalar_add(dst, pq, qb_sb[:, mt // 2:mt // 2 + 1])
                        else:
                            nc.vector.tensor_copy(out=dst, in_=pq)

                    def v_block(b):
                        # token-major V for one batch, psum borrowed from ps_qk
                        for jc in range(2):
                            m = 128 if jc == 0 else N - 128
                            tok0 = b * N + jc * 128
                            tc_, to_ = tok0 // QK_TW, tok0 % QK_TW
                            for nt in range(2):
                                pv = ps_qk.tile([128, QK_TW], F32, name="psqk", tag="psqk")
                                for kt in range(KT):
                                    nc.tensor.matmul(
                                        pv[0:m, 0:C // 2],
                                        lhsT=xT_sb[:, tc_, kt, to_:to_ + m],
                                        rhs=wv_sb[:, kt, nt * (C // 2):(nt + 1) * (C // 2)],
                                        start=(kt == 0),
                                        stop=(kt == KT - 1 and not has_v_bias),
                                    )
                                if has_v_bias:
                                    nc.tensor.matmul(
                                        pv[0:m, 0:C // 2],
                                        lhsT=ones_row[:, tok0:tok0 + m],
                                        rhs=vb_sb2[:, nt * (C // 2):(nt + 1) * (C // 2)],
                                        start=False, stop=True,
                                    )
                                evac_copy(
                                    V_sb[0:m, b * 2 + jc, nt * 6:(nt + 1) * 6, 0:HD],
                                    pv[0:m, 0:C // 2].rearrange("p (g f) -> p g f", g=KT),
                                )

                    def attn_scores(hp, b, mul_eng):
                        q0 = b * N
                        # Both heads' scores in one 2-bank PSUM tile (head hh at
                        # columns hh*512 + [0, 394)) so one bank-hopping AP
                        # covers the pair in a single EXP. Even/odd heads live
                        # at base partitions 0/64, so their k=64 score matmuls
                        # occupy disjoint PE row groups and overlap.
                        ps_s2 = ps_s_pool.tile([128, 1024], F32, name="ps_s2", tag="ps_s2")
                        for jc in range(2):
                            for hh in range(2):
                                h = hp * 2 + hh
                                pbase = (h % 2) * 64
                                nc.tensor.matmul(
                                    ps_s2[:, hh * 512 + jc * N:hh * 512 + (jc + 1) * N],
                                    lhsT=qkT_sb[pbase:pbase + 64, 2 * hp + 1,
                                                q0 + jc * 128:q0 + jc * 128 + 128],
                                    rhs=qkT_sb[pbase:pbase + 64, 2 * hp, q0:q0 + N],
                                    start=(jc == 0), stop=(jc == 1),
                                )
                        expS2 = expwork.tile([128, 4 * N], BF16, tag="expS2")
                        nc.scalar.activation(
                            expS2.rearrange("p (g w) -> p g w", w=2 * N),
                            ps_s2.rearrange("p (g w) -> p g w", w=512)[:, :, 0:2 * N],
                            mybir.ActivationFunctionType.Exp, scale=SCALE,
                        )
                        expST2 = expwork.tile([128, 4 * N], BF16, tag="expST2")
                        mul_eng.tensor_mul(expST2, expS2, expb_sb[:, hp, :])
                        return expST2

                    def attn_av(hp, b, av, expST2):
                        for hh in range(2):
                            h = hp * 2 + hh
                            for jc in range(2):
                                jm = 128 if jc == 0 else N - 128
                                nc.tensor.matmul(
                                    av[:, hh * N:(hh + 1) * N],
                                    lhsT=V_sb[0:jm, b * 2 + jc, h, :],
                                    rhs=expST2[0:jm, hh * 2 * N + jc * N:hh * 2 * N + (jc + 1) * N],
                                    start=(hh == 0 and jc == 0),
                                    stop=(hh == 1 and jc == 1),
                                )

                    def attn_back2(hp, b, av2):
                        # one copy/recip/mul set covers pairs (hp,b) and (hp,b+1)
                        denom2 = normwork.tile([64, 2, 2 * N], F32, tag="denom")
                        nc.scalar.copy(out=denom2, in_=av2[64:128, :, 0:2 * N])
                        recipB2 = normwork.tile([64, 2, 2 * N], F32, tag="recipB")
                        nc.vector.reciprocal_approx_fast(recipB2, denom2)
                        for hh in range(2):
                            h = hp * 2 + hh
                            dst = aoT_sb[(h % 2) * 64:(h % 2) * 64 + 64, h // 2,
                                         b * N:(b + 2) * N]
                            nc.vector.tensor_mul(
                                dst.rearrange("p (n w) -> p n w", n=2),
                                av2[0:HD, :, hh * N:(hh + 1) * N],
                                recipB2[:, :, hh * N:(hh + 1) * N],
                            )

                    out_dma = [nc.sync, nc.scalar]

                    def proj_mtile(mt):
                        rows = min(128, NTOK - mt * 128)
                        stage = outstage.tile([128, C], F32 if OUTF32 else BF16, tag="stage")
                        # nt-outer: the two psum tiles retire alternately so the
                        # 2-deep ring never stalls the PE between m-tiles
                        for nt in range(2):
                            ps = ps_qk.tile([128, QK_TW], F32, name="psqk", tag="psqk")
                            for kt in range(KT):
                                nc.tensor.matmul(
                                    ps[:, 0:C // 2],
                                    lhsT=aoT_sb[:, kt, mt * 128:(mt + 1) * 128],
                                    rhs=wp_sb[:, kt, nt * (C // 2):(nt + 1) * (C // 2)],
                                    start=(kt == 0),
                                    stop=(kt == KT - 1 and not has_p_bias),
                                )
                            if has_p_bias:
                                nc.tensor.matmul(
                                    ps[:, 0:C // 2],
                                    lhsT=ones_row[:, mt * 128:(mt + 1) * 128],
                                    rhs=pb_sb[:, nt * (C // 2):(nt + 1) * (C // 2)],
                                    start=False, stop=True,
                                )
                            if nt == 0:
                                nc.scalar.copy(out=stage[:, 0:C // 2], in_=ps[:, 0:C // 2])
                            else:
                                nc.vector.tensor_copy(out=stage[:, C // 2:C], in_=ps[:, 0:C // 2])
                        out_dma[0 if OUTF32 else mt % 2].dma_start(
                            out=out_d.ap()[mt * 128:mt * 128 + rows, :], in_=stage[0:rows, :]
                        )

                    # ---- filler unit scheduler ------------------------------
                    # Units are dense work (qk pieces, V blocks, proj m-tiles,
                    # deferred memsets) emitted between attention pairs so the
                    # FIFO PE queue never head-blocks on the softmax chain.
                    units = []
                    qk_done = set()

                    def qk_unit(mt, nt):
                        return ("qk", (mt, nt), lambda: qk_piece(mt, nt))

                    def v_unit(b):
                        return ("v", b, lambda: v_block(b))

                    def proj_unit(mt):
                        return ("proj", mt, lambda: proj_mtile(mt))

                    def run_unit(u):
                        if u[0] == "qk":
                            qk_done.add(u[1])
                        u[2]()

                    def filler_step(n=2):
                        for _ in range(n):
                            if units:
                                run_unit(units.pop(0))

                    def ensure_qk(mt, nt):
                        while (mt, nt) not in qk_done and units:
                            run_unit(units.pop(0))

                    inflight = []   # av2 groups awaiting back2
                    pend = []       # scored pairs awaiting their AV matmuls
                    pair_n = [0]
                    # 3:1 gpsimd:vector for the exp(bias) multiply (SBUF-only op)
                    mul_cycle = [nc.gpsimd, nc.gpsimd, nc.vector, nc.gpsimd]

                    def flush_av():
                        # emit AV matmuls for the two OLDEST scored pairs —
                        # their expb-muls got a full slot of slack, so the
                        # FIFO PE queue never head-blocks on the softmax chain
                        (h0, b0, e0), (h1, b1, e1) = pend.pop(0), pend.pop(0)
                        av2 = ps_av_pool.tile([128, 2, 512], F32, name="av2", tag="av2")
                        attn_av(h0, b0, av2[:, 0, 0:2 * N], e0)
                        attn_av(h1, b1, av2[:, 1, 0:2 * N], e1)
                        inflight.append((h0, b0, av2))

                    def flush_back():
                        hp, b, av2 = inflight.pop(0)
                        attn_back2(hp, b, av2)
                        if hp == H // 2 - 1:
                            units.extend(proj_unit(mt) for mt in _PROJ_OF_GROUP[b // 2])

                    # PE warm-up while DMAs land: keeps HAM at 2.4 GHz
                    if WARMUP:
                        pw = ps_qk.tile([128, QK_TW], F32, name="psqk", tag="psqk")
                        for w in range(WARMUP):
                            nc.tensor.matmul(
                                pw, lhsT=junk_sb[:, 0:128], rhs=junk_sb[:, 0:QK_TW],
                                start=(w == 0), stop=(w == WARMUP - 1),
                            )

                    # ---- prologue: group 0's first head pair + V -----------
                    for mt in range(2):
                        qk_piece(mt, 0)
                        qk_done.add((mt, 0))
                    v_block(0)
                    v_block(1)
                    units.extend(qk_unit(mt, 0) for mt in range(2, 12))

                    # ---- main loop: batch-pair groups ----------------------
                    for g in range(QK_NT):
                        if g + 1 < QK_NT:
                            nxt = [qk_unit(mt, g + 1) for mt in range(4)]
                            nxt.append(v_unit(2 * (g + 1)))
                            nxt.append(v_unit(2 * (g + 1) + 1))
                            nxt += [qk_unit(mt, g + 1) for mt in range(4, 12)]
                            units.extend(nxt)
                        for hp in range(H // 2):
                            for fire in _gates.get((g, hp), ()):
                                fire()
                            ensure_qk(2 * hp, g)
                            ensure_qk(2 * hp + 1, g)
                            for db in range(2):
                                mul_eng = mul_cycle[pair_n[0] % len(mul_cycle)]
                                pair_n[0] += 1
                                pend.append((hp, 2 * g + db,
                                             attn_scores(hp, 2 * g + db, mul_eng)))
                            if len(inflight) >= 2:
                                flush_back()
                            filler_step(2)
                            if len(pend) >= 4:
                                flush_av()

                    # ---- drain --------------------------------------------
                    while len(pend) >= 2:
                        flush_av()
                        if len(inflight) >= 2:
                            flush_back()
                            filler_step(1)
                    while inflight:
                        flush_back()
                        filler_step(2)
                    while units:
                        run_unit(units.pop(0))

    nc.compile()
    return nc


def _bf16(a):
    return np.ascontiguousarray(a).astype(ml_dtypes.bfloat16)


def _host_prep(qkv_w, proj_w, rel_pos_table, rel_pos_index):
    """Shared (core-independent) SBUF images."""
    # exp of the transposed per-head bias, laid out as the kernel's score
    # tiles, two heads (one pair) side by side: head hh of pair hp occupies
    # columns hh*2N + [0, 2N): j-chunk0 rows 0:128, j-chunk1 rows 0:69 with
    # rows 69:128 zeroed (kills padded key rows).
    bias = rel_pos_table[rel_pos_index.reshape(-1)].reshape(N, N, H)  # [i, j, h]
    expb = np.zeros((128, H // 2, 4 * N), dtype=np.float32)
    eb = np.exp(bias)
    for h in range(H):
        ebT = eb[:, :, h].T  # [j, i]
        base = (h % 2) * 2 * N
        expb[:, h // 2, base:base + N] = ebT[0:128, :]
        expb[0:N - 128, h // 2, base + N:base + 2 * N] = ebT[128:N, :]
    expb16 = _bf16(expb)

    # wqk image [p, mt, kt, c], m-tile order [Q0,K0,Q1,K1,...]
    wqkT = np.ascontiguousarray(qkv_w[0:2 * C].T)            # [768, 1536]
    img = wqkT.reshape(KT, 128, 2, KT, 128).transpose(1, 3, 2, 0, 4)  # p,hp,s,kt,c
    wqk16 = _bf16(img.reshape(128, 12, KT, 128))

    wv16 = _bf16(qkv_w[2 * C:3 * C].T.reshape(KT, 128, C).transpose(1, 0, 2))
    wp16 = _bf16(proj_w.T.reshape(KT, 128, C).transpose(1, 0, 2))
    return wqk16, wv16, wp16, expb16


def _core_in_map(x, shared, core, has_q, has_v, has_p,
                 q_bias=None, v_bias=None, proj_b=None):
    wqk16, wv16, wp16, expb16 = shared
    xs = x[core * B_LOC:(core + 1) * B_LOC]                  # [8, 197, 768]
    # chunk-major image [p, chunk, kt, t'] so each chunk DMA is contiguous
    xT = _bf16(xs.reshape(NTOK, C).T.reshape(KT, 128, QK_NT, QK_TW)
               .transpose(1, 2, 0, 3))
    m = {"xT": xT, "wqk": wqk16, "wv": wv16, "wp": wp16, "expb": expb16}
    if has_q:
        m["qb"] = np.ascontiguousarray(q_bias.reshape(KT, 128).T)
    if has_v:
        m["vb"] = _bf16(v_bias.reshape(1, C))
    if has_p:
        m["pb"] = _bf16(proj_b.reshape(1, C))
    return m


def kernel(x, qkv_w, q_bias, v_bias, rel_pos_table, proj_w, proj_b, rel_pos_index):
    x = np.asarray(x, dtype=np.float32)
    qkv_w = np.asarray(qkv_w, dtype=np.float32)
    q_bias = np.asarray(q_bias, dtype=np.float32)
    v_bias = np.asarray(v_bias, dtype=np.float32)
    rel_pos_table = np.asarray(rel_pos_table, dtype=np.float32)
    proj_w = np.asarray(proj_w, dtype=np.float32)
    proj_b = np.asarray(proj_b, dtype=np.float32)
    rel_pos_index = np.asarray(rel_pos_index)

    has_q = bool(np.any(q_bias != 0))
    has_v = bool(np.any(v_bias != 0))
    has_p = bool(np.any(proj_b != 0))

    key = (has_v, has_p, has_q)
    if key not in _CACHE:
        _CACHE[key] = _build(*key)
    nc = _CACHE[key]

    shared = _host_prep(qkv_w, proj_w, rel_pos_table, rel_pos_index)
    in_maps = [
        _core_in_map(x, shared, c, has_q, has_v, has_p, q_bias, v_bias, proj_b)
        for c in range(N_CORES)
    ]

    res = bass_utils.run_bass_kernel_spmd(nc, in_maps, core_ids=list(range(N_CORES)))
    out = np.empty((B, N, C), dtype=np.float32)
    for c in range(N_CORES):
        out[c * B_LOC:(c + 1) * B_LOC] = np.asarray(
            res.results[c]["out"], dtype=np.float32).reshape(B_LOC, N, C)
    return out
